# revision 27
# baseline (speedup 1.0000x reference)
"""Trainium2 Bass kernel for nn_BasicTransformerBlock (self-attn + cross-attn
+ GEGLU FF, dim=1024, heads=16, seq=4096, ctx=77).

Strategy (8 NeuronCores), v2:
 - Sequence-parallel: each core owns 512 tokens end-to-end, activations kept
   transposed on-chip ([channel, token]).
 - K and augmented-V for self-attention are fp8 (e4m3); one fused AllGather
   ships both (half the wire bytes of the bf16 baseline). The AG bubble is
   filled with the Q projection, the cross-attn K2/V2 projections, and the
   "diagonal" score+exp prefill (own-rank key tiles from SBUF).
 - Softmax without max-subtraction. exp is split across ScalarE (table exp,
   fp8 out) and VectorE (integer Schraudolph: bits = round(s*8/ln2 + 8*(7-C))
   written as uint8, bit-identical to fp8e4m3) - the two engines each handle
   ~half of the 33M exps, which was the baseline's critical path.
 - P^T and V both fp8 => AV matmuls run in DoubleRow mode (256-deep virtual
   contraction), halving the attention AV matmul count. The V rows carry a
   trailing ones column per head so softmax denominators fall out of the AV
   matmul for free.
 - All row broadcasts (LN scale/shift, softmax 1/z) are K=1/K=2 matmuls on
   TensorE instead of DRAM-bounce broadcast DMAs.
 - Weights stay bf16 (fp8 weights push rel-err to ~2e-2). Weight DMAs are
   batched in 2-m-tile pairs and issued on the scalar HWDGE queue so they
   never block attention-critical loads on the sync queue; FF1/FF2 tiles are
   prefetched during attention.
Host gathers the 8 transposed output shards and transposes back.
"""
import numpy as np
import ml_dtypes
from contextlib import ExitStack

import concourse.bass as bass
import concourse.tile as tile
import concourse.mybir as mybir
from concourse.bass_utils import run_bass_kernel_spmd


# --- inlined BIR sync-wait legalizer (toolchain accepts max 1 wait/inst) ---
import json as _json


def _legalize_bir_json(raw, max_waits=1):
    d = _json.loads(raw)
    ctr = 0
    for f in d.get("functions", []):
        for bb in f.get("blocks", []):
            out = []
            for ins in bb.get("instructions", []):
                si = ins.get("sync_info")
                if si:
                    waits = si.get("on_wait") or []
                    if len(waits) > max_waits:
                        extra, keep = waits[:-max_waits], waits[-max_waits:]
                        for w in extra:
                            ctr += 1
                            out.append({
                                "debug": ins.get("debug", 0),
                                "engine": ins["engine"],
                                "ins": [],
                                "outs": [],
                                "name": f"waitfix-{ctr}",
                                "opcode": "EventSemaphore",
                                "sync_info": {"on_update": [], "on_wait": [w]},
                            })
                        si["on_wait"] = keep
                    ups = si.get("on_update") or []
                    if len(ups) > 1:
                        raise AssertionError(
                            f"instruction {ins.get('name')} has {len(ups)} updates")
                out.append(ins)
            bb["instructions"] = out
    return _json.dumps(d).encode()


def _install_legalizer(max_waits=1):
    import concourse.bass as _bassmod

    if getattr(_bassmod.Bass, "_legalize_installed", False):
        return
    orig = _bassmod.Bass.to_json_bytes

    def patched(self):
        return _legalize_bir_json(orig(self), max_waits=max_waits)

    _bassmod.Bass.to_json_bytes = patched
    _bassmod.Bass._legalize_installed = True


_install_legalizer()


def _install_ldwopt():
    import concourse.bass_utils as _bu

    if getattr(_bu, "_ldwopt_patched", False):
        return
    _orig_rc = _bu.run_command

    def _rc(cmd, **kw):
        try:
            cmd = ["--enable-ldw-opt=true" if c == "--enable-ldw-opt=false"
                   else c for c in cmd]
        except TypeError:
            pass
        return _orig_rc(cmd, **kw)

    _bu.run_command = _rc
    _bu._ldwopt_patched = True


# NOTE: --enable-ldw-opt=true rejects DoubleRow InstLdweights; left disabled.
# _install_ldwopt()

F32 = mybir.dt.float32
F32R = mybir.dt.float32r
BF16 = mybir.dt.bfloat16
F8 = mybir.dt.float8e4
U8 = mybir.dt.uint8
AF = mybir.ActivationFunctionType
OP = mybir.AluOpType
DR = mybir.MatmulPerfMode.DoubleRow

DIM = 1024
HEADS = 16
D = 64
CTX = 768
FF = 4096
T = 4096
NCORES = 8
TO = T // NCORES          # 512 own tokens per core
PAIRS = HEADS // 2        # 8 head pairs
CKT = DIM // 128          # 8 contraction tiles over DIM
CKT_CTX = CTX // 128      # 6 contraction tiles over CTX
TCX = 77
TCXP = 80                 # ctx tokens padded
SCALE = D ** -0.5
EPS = 1e-5

# V augmented row layout (fp8): per pair a 160-col block:
#   [0:64)  V head A   [64] ones A   [65:80) pad
#   [80:144) V head B  [144] ones B  [145:160) pad
PB = 160                     # pair block width
V_ROWP = PAIRS * PB          # 1280
K_ELEMS = DIM * TO           # 524288 fp8 bytes
V_ELEMS = TO * V_ROWP        # 655360
AG_ELEMS = K_ELEMS + V_ELEMS

# Schraudolph fp8 exp constants (validated bit-exact vs HW probe)
LOG2E = 1.4426950408889634
SCH_C = 0.0430
SCH_A8 = 8.0 * LOG2E
SCH_B8 = 8.0 * (7.0 - SCH_C)


def _ap(tensor_ap, offset, steps):
    """Raw AP view on a (flat) dram tensor: steps = [[step, count], ...]."""
    return bass.AP(tensor=tensor_ap.tensor, offset=tensor_ap.offset + offset,
                   ap=list(steps))


def build_nc(fake_ag=False):
    nc = bass.Bass(trn_type="TRN2")

    # ---- dram tensors ----------------------------------------------------
    xT = nc.dram_tensor("xT", [DIM, TO], F32, kind="ExternalInput")
    ctxT = nc.dram_tensor("ctxT", [CTX, TCXP], BF16, kind="ExternalInput")

    def w_in(name, shape=None, dt=BF16, shape_=None):
        return nc.dram_tensor(name, list(shape if shape is not None else shape_),
                              dt, kind="ExternalInput")

    # paired m-tile layouts: [nm/2, 128, 2, nkt, 128]
    wq1t = w_in("wq1t", (4, 128, 2, CKT, 128))
    wk1t = w_in("wk1t", (4, 128, 2, CKT, 128))
    wv1t = w_in("wv1t", (2, 128, CKT, 512))
    o1t = w_in("o1t", (4, 128, 2, CKT, 128))
    wq2t = w_in("wq2t", (4, 128, 2, CKT, 128))
    k2t = w_in("k2t", (4, 128, 2, CKT_CTX, 128))
    v2t = w_in("v2t", (2, 128, CKT_CTX, 512))
    o2t = w_in("o2t", (4, 128, 2, CKT, 128))
    ff1t = w_in("ff1t", (32, 128, 2, CKT, 128))   # [i] = (gate 32+i, a i)
    ff2t = w_in("ff2t", (8, 128, FF // 128, 128))

    sel16d = w_in("sel16", dt=BF16, shape_=(16, 8, 128))
    qb1c = w_in("qb1c", dt=F32, shape_=(128, 8))
    kb1c = w_in("kb1c", dt=F32, shape_=(128, 8))
    vb1r = w_in("vb1r", dt=BF16, shape_=(1, DIM))
    o1bc = w_in("o1bc", dt=F32, shape_=(128, 8))
    qb2c = w_in("qb2c", dt=F32, shape_=(128, 8))
    o2bc = w_in("o2bc", dt=F32, shape_=(128, 8))
    fb1c = w_in("fb1c", dt=F32, shape_=(128, 64))
    padmask = w_in("padmask", dt=F32, shape_=(128, 16))
    ff2bc = w_in("ff2bc", dt=F32, shape_=(128, 8))

    outT = nc.dram_tensor("outT", [DIM, TO], F32, kind="ExternalOutput")

    with tile.TileContext(nc) as tc, ExitStack() as top:
        dram = top.enter_context(tc.tile_pool(name="dram", bufs=1, space="DRAM"))
        p_const = top.enter_context(tc.tile_pool(name="p_const", bufs=1))

        # ---- constants ---------------------------------------------------
        ones_col_f = p_const.tile([128, 1], F32, name="ones_col_f")
        nc.vector.memset(ones_col_f[:], 1.0)
        ones_col = p_const.tile([128, 1], F32R, name="ones_col")
        nc.scalar.copy(ones_col[:], ones_col_f[:])
        ones_row_bf = p_const.tile([1, 128], BF16, name="ones_row_bf")
        nc.vector.memset(ones_row_bf[:], 1.0)
        # sel16[:, p, :]: K=16 selector that broadcasts 1/z rows 2p (to
        # partitions 0:64) and 2p+1 (to 64:128)
        sel16 = p_const.tile([16, 8, 128], BF16, name="sel16")
        nc.sync.dma_start(out=sel16, in_=sel16d.ap())
        padones = p_const.tile([128, 16], F32, name="padones")
        nc.sync.dma_start(out=padones, in_=padmask.ap())
        eps_row = p_const.tile([1, 1], F32, name="eps_row")
        nc.vector.memset(eps_row[:], EPS)

        def bias_tile(name, dram_t, cols):
            t = p_const.tile([128, cols], F32, name=name)
            nc.sync.dma_start(out=t, in_=dram_t.ap())
            return t

        qb1 = bias_tile("qb1", qb1c, 8)
        kb1 = bias_tile("kb1", kb1c, 8)
        o1b = bias_tile("o1b", o1bc, 8)
        qb2 = bias_tile("qb2", qb2c, 8)
        o2b = bias_tile("o2b", o2bc, 8)
        fb1 = bias_tile("fb1", fb1c, 64)
        ff2b = bias_tile("ff2b", ff2bc, 8)
        # vb1 broadcast via K=1 matmul at V-evac time needs [128, DIM] view;
        # build it once into SBUF from a psum broadcast.
        vb1row = p_const.tile([1, DIM], BF16, name="vb1row")
        nc.sync.dma_start(out=vb1row, in_=vb1r.ap())
        vb1bc = p_const.tile([128, DIM], F32, name="vb1bc")
        with ExitStack() as st0:
            psb0 = st0.enter_context(tc.tile_pool(name="psb0", bufs=1, space="PSUM"))
            pv = psb0.tile([128, 512], F32, name="pv", tag="pv")
            for half in range(2):
                nc.tensor.matmul(pv[:], ones_row_bf[:],
                                 vb1row[:, half * 512:(half + 1) * 512],
                                 start=True, stop=True)
                nc.vector.tensor_copy(vb1bc[:, half * 512:(half + 1) * 512], pv[:])
        ctx_sb = []
        for i in range(CKT_CTX):
            t = p_const.tile([128, TCXP], BF16, name=f"ctxsb{i}")
            nc.sync.dma_start(out=t, in_=ctxT.ap()[i * 128:(i + 1) * 128, :])
            ctx_sb.append(t)

        # exp engine alternation counter
        exp_state = {"i": 0}

        def exp_to_fp8(out_ap, in_ap):
            """exp of a PSUM tile into an fp8 SBUF AP; alternates ACT/DVE."""
            if exp_state["i"] % 2 == 0:
                nc.scalar.activation(out_ap, in_ap, AF.Exp)
            else:
                nc.vector.tensor_scalar(out_ap.bitcast(U8), in_ap,
                                        SCH_A8, SCH_B8, op0=OP.mult, op1=OP.add)
            exp_state["i"] += 1

        def exp_split(out4, lt, pss):
            """exp of one [128,2,TO] score tile: head A on ScalarE (table),
            head B on VectorE (integer Schraudolph). Half the latency of a
            single-engine pass, so score PSUM frees faster."""
            nc.scalar.activation(out4[:, lt, 0, :], pss[:, 0, :], AF.Exp)
            nc.vector.tensor_scalar(out4[:, lt, 1, :].bitcast(U8), pss[:, 1, :],
                                    SCH_A8, SCH_B8, op0=OP.mult, op1=OP.add)

        def exp_to_bf16(out_ap, in_ap):
            if exp_state["i"] % 2 == 0:
                nc.scalar.activation(out_ap, in_ap, AF.Exp)
            else:
                nc.vector.tensor_scalar(out_ap.bitcast(mybir.dt.uint16), in_ap,
                                        128.0 * LOG2E, 128.0 * (127.0 - SCH_C),
                                        op0=OP.mult, op1=OP.add)
            exp_state["i"] += 1

        # ---- helpers -----------------------------------------------------
        def layernorm(xtiles, h_pool, tag, out_dtype=BF16):
            """xtiles: 8 sbuf tiles [128, TO] F32R -> 8 out tiles [128,TO]."""
            with ExitStack() as ln:
                work = ln.enter_context(tc.tile_pool(name=f"lnw_{tag}", bufs=2))
                rows = ln.enter_context(tc.tile_pool(name=f"lnr_{tag}", bufs=1))
                ps = ln.enter_context(tc.tile_pool(name=f"lnp_{tag}", bufs=1,
                                                   space="PSUM"))
                ps_s = ps.tile([1, TO], F32, name=f"pss_{tag}", tag="s")
                ps_q = ps.tile([1, TO], F32, name=f"psq_{tag}", tag="q")
                for i in range(8):
                    sq = work.tile([128, TO], F32R, name=f"sq_{tag}", tag="sq")
                    nc.gpsimd.tensor_tensor(sq[:], xtiles[i].bitcast(F32),
                                            xtiles[i].bitcast(F32), op=OP.mult)
                    nc.tensor.matmul(ps_s[:], ones_col[:], xtiles[i][:],
                                     start=(i == 0), stop=(i == 7))
                    nc.tensor.matmul(ps_q[:], ones_col[:], sq[:],
                                     start=(i == 0), stop=(i == 7))
                mu = rows.tile([1, TO], F32, name=f"mu_{tag}")
                nc.vector.tensor_scalar(mu[:], ps_s[:], 1.0 / DIM, None, op0=OP.mult)
                m2 = rows.tile([1, TO], F32, name=f"m2_{tag}")
                nc.vector.tensor_scalar(m2[:], ps_q[:], 1.0 / DIM, None, op0=OP.mult)
                var = rows.tile([1, TO], F32, name=f"var_{tag}")
                nc.vector.tensor_tensor(var[:], mu[:], mu[:], op=OP.mult)
                nc.vector.tensor_tensor(var[:], m2[:], var[:], op=OP.subtract)
                # 1/sqrt(v+eps) = exp(-0.5*ln(v+eps)); table ops beat the
                # 3.3us single-row DVE reciprocal
                lnv = rows.tile([1, TO], F32, name=f"lnv_{tag}")
                nc.scalar.activation(lnv[:], var[:], AF.Ln, bias=eps_row[:])
                ra = rows.tile([1, TO], F32, name=f"ra_{tag}")
                nc.scalar.activation(ra[:], lnv[:], AF.Exp, scale=-0.5)
                rb = rows.tile([1, TO], F32, name=f"rb_{tag}")
                nc.vector.scalar_tensor_tensor(rb[:], mu[:], -1.0, ra[:],
                                               op0=OP.mult, op1=OP.mult)
                rab = rows.tile([1, TO], BF16, name=f"rab_{tag}")
                nc.scalar.copy(rab[:], ra[:])
                rbb = rows.tile([1, TO], BF16, name=f"rbb_{tag}")
                nc.scalar.copy(rbb[:], rb[:])
                ps_a = ps.tile([128, TO], F32, name=f"psa_{tag}", tag="a")
                ps_b = ps.tile([128, TO], F32, name=f"psb_{tag}", tag="b")
                nc.tensor.matmul(ps_a[:], ones_row_bf[:], rab[:],
                                 start=True, stop=True)
                nc.tensor.matmul(ps_b[:], ones_row_bf[:], rbb[:],
                                 start=True, stop=True)
                bsb = rows.tile([128, TO], F32, name=f"bsb_{tag}")
                nc.vector.tensor_copy(bsb[:], ps_b[:])
                out = []
                for i in range(8):
                    tmp = work.tile([128, TO], F32, name=f"tmp_{tag}", tag="tmp")
                    nc.vector.tensor_tensor(tmp[:], xtiles[i].bitcast(F32),
                                            ps_a[:], op=OP.mult)
                    h = h_pool.tile([128, TO], out_dtype, name=f"h_{tag}{i}")
                    nc.gpsimd.tensor_tensor(h[:], tmp[:], bsb[:], op=OP.add)
                    out.append(h)
                return out

        def proj_T(wdram, rhs_tiles, bias, out_pool, tag, nkt=CKT,
                   out_dtype=BF16, residual=None, res_bias=None, nmb=4,
                   store_dram=None):
            """Paired m-tile projection. wdram: [nmb, 128, 2, nkt, 128]."""
            outs = []
            with ExitStack() as st:
                wp = st.enter_context(tc.tile_pool(name=f"wp_{tag}", bufs=2))
                ps = st.enter_context(tc.tile_pool(name=f"ps_{tag}", bufs=2,
                                                   space="PSUM"))
                wtiles = []
                for mb in range(min(2, nmb)):
                    wm = wp.tile([128, 2, nkt, 128], BF16, name=f"wm_{tag}", tag="w")
                    nc.sync.dma_start(out=wm, in_=wdram.ap()[mb])
                    wtiles.append(wm)
                for mb in range(nmb):
                    wm = wtiles[mb]
                    for j in range(2):
                        m = 2 * mb + j
                        psy = ps.tile([128, TO], F32, name=f"psy_{tag}", tag="y")
                        for kt in range(nkt):
                            nc.tensor.matmul(psy[:], wm[:, j, kt, :],
                                             rhs_tiles[kt][:],
                                             start=(kt == 0), stop=(kt == nkt - 1))
                        o = out_pool.tile([128, TO], out_dtype, name=f"o_{tag}{m}")
                        if residual is not None:
                            nc.vector.scalar_tensor_tensor(
                                o[:], psy[:], res_bias[:, m:m + 1],
                                residual[m].bitcast(F32), op0=OP.add, op1=OP.add)
                        elif bias is not None:
                            nc.vector.tensor_scalar(o[:], psy[:], bias[:, m:m + 1],
                                                    None, op0=OP.add)
                        else:
                            nc.vector.tensor_copy(o[:], psy[:])
                        if store_dram is not None:
                            nc.sync.dma_start(
                                out=_ap(store_dram, m * 128 * TO,
                                        [[TO, 128], [1, TO]]),
                                in_=o[:])
                        outs.append(o)
                    if mb + 2 < nmb:
                        wn = wp.tile([128, 2, nkt, 128], BF16, name=f"wm_{tag}",
                                     tag="w")
                        nc.sync.dma_start(out=wn, in_=wdram.ap()[mb + 2])
                        wtiles.append(wn)
            return outs

        # ---- AG buffer (fused K + augmented V, fp8) ----------------------
        agkv_in = dram.tile([AG_ELEMS], F8, name="agkv_in")
        agkv_out = dram.tile([NCORES * AG_ELEMS], F8, name="agkv_out",
                             addr_space="Local" if fake_ag else "Shared")

        # ================= phase A: LN1 + K/V/Q projections ===============
        p_xT = top.enter_context(tc.tile_pool(name="p_xT", bufs=1))
        p_QT = top.enter_context(tc.tile_pool(name="p_QT", bufs=1))
        p_OT = top.enter_context(tc.tile_pool(name="p_OT", bufs=1))
        p_kv = top.enter_context(tc.tile_pool(name="p_kv", bufs=1))

        xtiles = []
        for i in range(8):
            t = p_xT.tile([128, TO], F32R, name=f"xT{i}")
            nc.sync.dma_start(out=t,
                              in_=xT.ap()[i * 128:(i + 1) * 128, :].bitcast(F32R))
            xtiles.append(t)

        with ExitStack() as phA:
            p_h1 = phA.enter_context(tc.tile_pool(name="p_h1", bufs=1))
            h1 = layernorm(xtiles, p_h1, "ln1")

            # K^T own (fp8) -> SBUF (for diagonal) + agkv_in rows [0 : DIM)
            ko = proj_T(wk1t, h1, kb1, p_kv, "k1", out_dtype=F8,
                        store_dram=agkv_in[:])

            # V own augmented (fp8) -> vag4 [128, 4, V_ROWP] + agkv_in
            vag4 = p_kv.tile([128, 4, PAIRS, PB], F8, name="vag4")
            with ExitStack() as stv:
                wvp = stv.enter_context(tc.tile_pool(name="wp_v1", bufs=1))
                ps = stv.enter_context(tc.tile_pool(name="ps_v1", bufs=2,
                                                    space="PSUM"))
                wv_sb = []
                for nb in range(2):
                    w = wvp.tile([128, CKT, 512], BF16, name=f"wv{nb}")
                    nc.sync.dma_start(out=w, in_=wv1t.ap()[nb])
                    wv_sb.append(w)
                for t4 in range(4):
                    for nb in range(2):
                        psv = ps.tile([128, 512], F32, name="psv", tag="v")
                        for kt in range(CKT):
                            nc.tensor.matmul(
                                psv[:], h1[kt][:, t4 * 128:(t4 + 1) * 128],
                                wv_sb[nb][:, kt, :],
                                start=(kt == 0), stop=(kt == CKT - 1))
                        # psv inner layout: 8 heads x 64; heads hh=0..7 map to
                        # pair pp = 4*nb + hh//2, half b = hh%2 at col 80b..80b+64
                        psv3 = psv.rearrange("p (g two e) -> p g two e", two=2, e=D)
                        vslice = vag4[:, t4, 4 * nb:4 * nb + 4, :]
                        for b in range(2):
                            nc.vector.tensor_tensor(
                                vslice[:, :, 80 * b:80 * b + D],
                                psv3[:, :, b, :],
                                vb1bc[:, nb * 512:(nb + 1) * 512].rearrange(
                                    "p (g two e) -> p g two e", two=2, e=D)[:, :, b, :],
                                op=OP.add)
                # ones columns (denominator rows) for both halves of all pairs
                for b in range(2):
                    nc.vector.memset(vag4[:, :, :, 80 * b + D:80 * b + D + 1], 1.0)
                nc.sync.dma_start(
                    out=_ap(agkv_in[:], K_ELEMS,
                            [[V_ROWP, 128], [128 * V_ROWP, 4], [1, V_ROWP]]),
                    in_=vag4[:])

            if fake_ag:
                for r in range(NCORES):
                    nc.sync.dma_start(
                        out=_ap(agkv_out[:], r * AG_ELEMS, [[1, AG_ELEMS]]),
                        in_=_ap(agkv_in[:], 0, [[1, AG_ELEMS]]))
            else:
                nc.gpsimd.collective_compute(
                    "AllGather", OP.bypass,
                    replica_groups=[list(range(NCORES))],
                    ins=[agkv_in[:]], outs=[agkv_out[:]])

            QT = proj_T(wq1t, h1, qb1, p_QT, "q1", out_dtype=F8)

        # ---- cross-attn K2/V2 from context (fills the AG bubble) ---------
        p_kv2 = top.enter_context(tc.tile_pool(name="p_kv2", bufs=1))
        K2T = []
        with ExitStack() as stk2:
            wp = stk2.enter_context(tc.tile_pool(name="wp_k2", bufs=2))
            ps = stk2.enter_context(tc.tile_pool(name="ps_k2", bufs=2, space="PSUM"))
            for mb in range(4):
                wm = wp.tile([128, 2, CKT_CTX, 128], BF16, name="wm_k2", tag="w")
                nc.sync.dma_start(out=wm, in_=k2t.ap()[mb])
                for j in range(2):
                    psy = ps.tile([128, TCXP], F32, name="psy_k2", tag="y")
                    for kt in range(CKT_CTX):
                        nc.tensor.matmul(psy[:], wm[:, j, kt, :], ctx_sb[kt][:],
                                         start=(kt == 0), stop=(kt == CKT_CTX - 1))
                    k2 = p_kv2.tile([128, TCXP], BF16, name=f"k2_{2 * mb + j}")
                    nc.vector.tensor_copy(k2[:], psy[:])
                    K2T.append(k2)

        V2_ROW = HEADS * (D + 1)  # 1040, ones at end of each 65-block
        v2ag = p_kv2.tile([TCXP, V2_ROW], BF16, name="v2ag")
        v2ag3 = v2ag.rearrange("p (h e) -> p h e", e=D + 1)
        with ExitStack() as stv2:
            wvp = stv2.enter_context(tc.tile_pool(name="wp_v2", bufs=1))
            ps = stv2.enter_context(tc.tile_pool(name="ps_v2", bufs=2, space="PSUM"))
            for nb in range(2):
                w = wvp.tile([128, CKT_CTX, 512], BF16, name=f"wv2_{nb}", tag="w")
                nc.sync.dma_start(out=w, in_=v2t.ap()[nb])
                psv = ps.tile([TCXP, 512], F32, name="psv2", tag="v")
                for kt in range(CKT_CTX):
                    nc.tensor.matmul(psv[:], ctx_sb[kt][:], w[:, kt, :],
                                     start=(kt == 0), stop=(kt == CKT_CTX - 1))
                nc.vector.tensor_copy(
                    v2ag3[:, nb * 8:(nb + 1) * 8, 0:D],
                    psv[:].rearrange("p (h e) -> p h e", e=D))
            nc.scalar.copy(v2ag3[:, :, D:D + 1], padones[0:TCXP, :].unsqueeze(2))

        # ================= phase B: self-attention ========================
        p_ff1w = top.enter_context(tc.tile_pool(name="p_ff1w", bufs=8))
        p_ff2w = top.enter_context(tc.tile_pool(name="p_ff2w", bufs=2))

        with ExitStack() as phB:
            ps_S = phB.enter_context(tc.tile_pool(name="ps_S", bufs=2, space="PSUM"))
            ps_AV = phB.enter_context(tc.tile_pool(name="ps_AV", bufs=1,
                                                   space="PSUM"))
            ps_BC = phB.enter_context(tc.tile_pool(name="ps_BC", bufs=2,
                                                   space="PSUM"))
            p_at = phB.enter_context(tc.tile_pool(name="p_at", bufs=2))
            p_pt = phB.enter_context(tc.tile_pool(name="p_pt", bufs=2))
            p_rb = phB.enter_context(tc.tile_pool(name="p_rb", bufs=2))

            def score_step(pss, lhsA, lhsB, qt):
                nc.tensor.matmul(pss[:, 0, :], lhsA, qt[0:64, :],
                                 start=True, stop=True, tile_position=(0, 0))
                nc.tensor.matmul(pss[:, 1, :], lhsB, qt[64:128, :],
                                 start=True, stop=True, tile_position=(64, 0))

            # --- bubble fill: own-token scores+exp warm-up from SBUF ------
            # (SPMD program cannot branch on rank, so these exps are
            # recomputed from the gathered buffer below; this pass just keeps
            # PE/ACT/DVE warm while the AllGather is in flight.)
            for p in range(PAIRS):
                pt = p_pt.tile([128, 4, 2, TO], F8, name="ptm", tag="pt")
                for lt in range(4):
                    pss = ps_S.tile([128, 2, TO], F32, name="pss", tag="s")
                    score_step(pss,
                               ko[p][0:64, lt * 128:(lt + 1) * 128],
                               ko[p][64:128, lt * 128:(lt + 1) * 128],
                               QT[p])
                    exp_split(pt, lt, pss)

            # --- prefetch first FF1 weight pairs on the scalar queue ------
            ff1_tiles = [None] * 32
            for i in range(8):
                wt = p_ff1w.tile([128, 2, CKT, 128], BF16, name="wff1", tag="w")
                nc.scalar.dma_start(out=wt, in_=ff1t.ap()[i])
                ff1_tiles[i] = wt

            # --- main loop ------------------------------------------------
            zsb = p_rb.tile([16, TO], F32, name="zsb", bufs=1)
            OT = []
            def load_pair(p):
                kpair = p_at.tile([128, NCORES, TO], F8, name="kpair", tag="kp")
                nc.gpsimd.dma_start(
                    out=kpair[:],
                    in_=_ap(agkv_out[:], p * 128 * TO,
                            [[TO, 128], [AG_ELEMS, NCORES], [1, TO]]))
                vpr = p_at.tile([128, NCORES, 4, PB], F8, name="vpr", tag="vp")
                for r in range(NCORES):
                    nc.gpsimd.dma_start(
                        out=vpr[:, r, :, :],
                        in_=_ap(agkv_out[:], r * AG_ELEMS + K_ELEMS + p * PB,
                                [[V_ROWP, 128], [128 * V_ROWP, 4], [1, PB]]))
                return kpair, vpr

            loaded = {0: load_pair(0)}
            for p in range(PAIRS):
                kpair, vpr = loaded.pop(p)
                if p + 1 < PAIRS:
                    loaded[p + 1] = load_pair(p + 1)

                psA = ps_AV.tile([65, 2, TO], F32, name="psA", tag="A")

                def av_step(r, ptm):
                    for i in range(2):
                        nc.tensor.matmul(
                            psA[:, 0, :],
                            vpr[:, r, 2 * i:2 * i + 2, 0:D + 1],
                            ptm[:, 2 * i:2 * i + 2, 0, :],
                            start=(r == 0 and i == 0),
                            stop=(r == NCORES - 1 and i == 1),
                            perf_mode=DR)
                        nc.tensor.matmul(
                            psA[:, 1, :],
                            vpr[:, r, 2 * i:2 * i + 2, 80:80 + D + 1],
                            ptm[:, 2 * i:2 * i + 2, 1, :],
                            start=(r == 0 and i == 0),
                            stop=(r == NCORES - 1 and i == 1),
                            perf_mode=DR)

                # software pipeline: scores+exp for rank r while AV of r-1
                # runs, so the in-order PE queue never stalls on exp results
                ptm_prev = None
                for r in range(NCORES):
                    ptm = p_pt.tile([128, 4, 2, TO], F8, name="ptm", tag="pt")
                    for lt in range(4):
                        pss = ps_S.tile([128, 2, TO], F32, name="pss", tag="s")
                        score_step(pss,
                                   kpair[0:64, r, lt * 128:(lt + 1) * 128],
                                   kpair[64:128, r, lt * 128:(lt + 1) * 128],
                                   QT[p])
                        exp_split(ptm, lt, pss)
                    if ptm_prev is not None:
                        av_step(r - 1, ptm_prev)
                    ptm_prev = ptm
                av_step(NCORES - 1, ptm_prev)

                # stash unnormalized AV + z rows; batched normalize below
                zrA = p_rb.tile([1, TO], F32, name="zrA", tag="zra")
                nc.vector.tensor_copy(zrA[:], psA[D:D + 1, 0, :])
                nc.gpsimd.dma_start(out=zsb[2 * p:2 * p + 1, :], in_=zrA[:])
                zrB = p_rb.tile([1, TO], F32, name="zrB", tag="zrb")
                nc.vector.tensor_copy(zrB[:], psA[D:D + 1, 1, :])
                nc.gpsimd.dma_start(out=zsb[2 * p + 1:2 * p + 2, :], in_=zrB[:])
                ot = p_OT.tile([128, TO], BF16, name=f"ot{p}")
                nc.vector.tensor_copy(ot[0:64, :], psA[0:D, 0, :])
                nc.vector.tensor_copy(ot[64:128, :], psA[0:D, 1, :])
                OT.append(ot)

            # batched softmax divide: one 16-lane reciprocal, then per-pair
            # K=16 selector matmul broadcast + in-place multiply
            zrec = p_rb.tile([16, TO], F32, name="zrec", bufs=1)
            nc.vector.reciprocal(zrec[:], zsb[:])
            zrecb = p_rb.tile([16, TO], BF16, name="zrecb", bufs=1)
            nc.scalar.copy(zrecb[:], zrec[:])
            for p in range(PAIRS):
                psbc = ps_BC.tile([128, TO], F32, name="psbc", tag="bc")
                nc.tensor.matmul(psbc[:], sel16[:, p, :], zrecb[:],
                                 start=True, stop=True)
                bcz = p_rb.tile([128, TO], BF16, name="bcz", tag="bcz")
                nc.vector.tensor_copy(bcz[:], psbc[:])
                nc.vector.tensor_tensor(OT[p][:], OT[p][:], bcz[:], op=OP.mult)

        # o1 projection + residual -> x2T
        p_x2 = top.enter_context(tc.tile_pool(name="p_x2", bufs=1))
        x2T = proj_T(o1t, OT, None, p_x2, "o1", residual=xtiles, res_bias=o1b,
                     out_dtype=F32R)

        # ================= phase C: cross-attention =======================
        p_x3 = top.enter_context(tc.tile_pool(name="p_x3", bufs=1))
        with ExitStack() as phC:
            p_Q2 = phC.enter_context(tc.tile_pool(name="p_Q2", bufs=1))
            p_OT2 = phC.enter_context(tc.tile_pool(name="p_OT2", bufs=1))

            with ExitStack() as stc:
                p_h2 = stc.enter_context(tc.tile_pool(name="p_h2", bufs=1))
                h2 = layernorm(x2T, p_h2, "ln2")
                Q2T = proj_T(wq2t, h2, qb2, p_Q2, "q2")

            with ExitStack() as stx:
                ps_S2 = stx.enter_context(tc.tile_pool(name="ps_S2", bufs=2,
                                                       space="PSUM"))
                ps_A2 = stx.enter_context(tc.tile_pool(name="ps_A2", bufs=1,
                                                       space="PSUM"))
                ps_B2 = stx.enter_context(tc.tile_pool(name="ps_B2", bufs=2,
                                                       space="PSUM"))
                p_pt2 = stx.enter_context(tc.tile_pool(name="p_pt2", bufs=2))
                p_rb2 = stx.enter_context(tc.tile_pool(name="p_rb2", bufs=2))
                zsb2 = p_rb2.tile([16, TO], F32, name="zsb2", bufs=1)
                OT2 = []
                for p in range(PAIRS):
                    pss = ps_S2.tile([TCXP, 2, TO], F32, name="pss2", tag="s")
                    nc.tensor.matmul(pss[:, 0, :], K2T[p][0:64, :], Q2T[p][0:64, :],
                                     start=True, stop=True, tile_position=(0, 0))
                    nc.tensor.matmul(pss[:, 1, :], K2T[p][64:128, :],
                                     Q2T[p][64:128, :],
                                     start=True, stop=True, tile_position=(64, 0))
                    pt = p_pt2.tile([TCXP, 2, TO], BF16, name="pt2", tag="pt")
                    exp_to_bf16(pt[:], pss[:])
                    psA = ps_A2.tile([D + 1, 2, TO], F32, name="psA2c", tag="A")
                    nc.tensor.matmul(psA[:, 0, :],
                                     v2ag[:, (2 * p) * (D + 1):(2 * p + 1) * (D + 1)],
                                     pt[:, 0, :], start=True, stop=True)
                    nc.tensor.matmul(psA[:, 1, :],
                                     v2ag[:, (2 * p + 1) * (D + 1):(2 * p + 2) * (D + 1)],
                                     pt[:, 1, :], start=True, stop=True)
                    zrA = p_rb2.tile([1, TO], F32, name="zrAc", tag="zra")
                    nc.vector.tensor_copy(zrA[:], psA[D:D + 1, 0, :])
                    nc.gpsimd.dma_start(out=zsb2[2 * p:2 * p + 1, :], in_=zrA[:])
                    zrB = p_rb2.tile([1, TO], F32, name="zrBc", tag="zrb")
                    nc.vector.tensor_copy(zrB[:], psA[D:D + 1, 1, :])
                    nc.gpsimd.dma_start(out=zsb2[2 * p + 1:2 * p + 2, :], in_=zrB[:])
                    ot = p_OT2.tile([128, TO], BF16, name=f"ot2_{p}")
                    nc.vector.tensor_copy(ot[0:64, :], psA[0:D, 0, :])
                    nc.scalar.copy(ot[64:128, :], psA[0:D, 1, :])
                    OT2.append(ot)

                zrec2 = p_rb2.tile([16, TO], F32, name="zrec2", bufs=1)
                nc.vector.reciprocal(zrec2[:], zsb2[:])
                zrecb2 = p_rb2.tile([16, TO], BF16, name="zrecb2", bufs=1)
                nc.scalar.copy(zrecb2[:], zrec2[:])
                for p in range(PAIRS):
                    psbc = ps_B2.tile([128, TO], F32, name="psbc2", tag="bc")
                    nc.tensor.matmul(psbc[:], sel16[:, p, :], zrecb2[:],
                                     start=True, stop=True)
                    bcz = p_rb2.tile([128, TO], BF16, name="bczc", tag="bcz")
                    nc.scalar.copy(bcz[:], psbc[:])
                    nc.vector.tensor_tensor(OT2[p][:], OT2[p][:], bcz[:],
                                            op=OP.mult)

            x3T = proj_T(o2t, OT2, None, p_x3, "o2", residual=x2T, res_bias=o2b,
                         out_dtype=F32R)

        # ================= phase D: GEGLU FF ==============================
        with ExitStack() as phD:
            p_hT = phD.enter_context(tc.tile_pool(name="p_hT", bufs=1))
            hT = []
            with ExitStack() as stf:
                p_h3 = stf.enter_context(tc.tile_pool(name="p_h3", bufs=1))
                h3 = layernorm(x3T, p_h3, "ln3")
                gp = stf.enter_context(tc.tile_pool(name="p_g", bufs=2))
                ps = stf.enter_context(tc.tile_pool(name="ps_ff1", bufs=3,
                                                    space="PSUM"))
                for i in range(32):
                    wt = ff1_tiles[i]
                    psg = ps.tile([128, TO], F32, name="psg", tag="p")
                    for kt in range(CKT):
                        nc.tensor.matmul(psg[:], wt[:, 0, kt, :], h3[kt][:],
                                         start=(kt == 0), stop=(kt == CKT - 1))
                    g = gp.tile([128, TO], F32, name="g", tag="g")
                    nc.scalar.activation(g[:], psg[:], AF.Gelu,
                                         bias=fb1[:, 32 + i:33 + i], scale=1.0)
                    psa = ps.tile([128, TO], F32, name="psa", tag="p")
                    for kt in range(CKT):
                        nc.tensor.matmul(psa[:], wt[:, 1, kt, :], h3[kt][:],
                                         start=(kt == 0), stop=(kt == CKT - 1))
                    h = p_hT.tile([128, TO], BF16, name=f"hT{i}")
                    nc.vector.scalar_tensor_tensor(h[:], psa[:], fb1[:, i:i + 1],
                                                   g[:], op0=OP.add, op1=OP.mult)
                    hT.append(h)
                    # stream in the next FF1 pair
                    if i + 8 < 32:
                        wt2 = p_ff1w.tile([128, 2, CKT, 128], BF16,
                                          name="wff1", tag="w")
                        nc.scalar.dma_start(out=wt2, in_=ff1t.ap()[i + 8])
                        ff1_tiles[i + 8] = wt2
                    if i == 0:
                        # kick off FF2 prefetch
                        ff2_tiles = []
                        for m in range(2):
                            w2 = p_ff2w.tile([128, FF // 128, 128], BF16,
                                             name="wff2", tag="w")
                            nc.scalar.dma_start(out=w2, in_=ff2t.ap()[m])
                            ff2_tiles.append(w2)

            with ExitStack() as stf2:
                outp = stf2.enter_context(tc.tile_pool(name="p_out", bufs=2))
                ps = stf2.enter_context(tc.tile_pool(name="ps_ff2", bufs=2,
                                                     space="PSUM"))
                for m in range(8):
                    wm = ff2_tiles[m % 2] if m < 2 else None
                    if m >= 2:
                        wm = p_ff2w.tile([128, FF // 128, 128], BF16,
                                         name="wff2", tag="w")
                        nc.scalar.dma_start(out=wm, in_=ff2t.ap()[m])
                    psy = ps.tile([128, TO], F32, name="psy_ff2", tag="y")
                    for kt in range(FF // 128):
                        nc.tensor.matmul(psy[:], wm[:, kt, :], hT[kt][:],
                                         start=(kt == 0), stop=(kt == FF // 128 - 1))
                    o = outp.tile([128, TO], F32, name="of", tag="of")
                    nc.vector.scalar_tensor_tensor(o[:], psy[:], ff2b[:, m:m + 1],
                                                   x3T[m].bitcast(F32),
                                                   op0=OP.add, op1=OP.add)
                    nc.sync.dma_start(out=outT.ap()[m * 128:(m + 1) * 128, :],
                                      in_=o[:])

    return nc


# ---------------------------------------------------------------------------
# host side
# ---------------------------------------------------------------------------
def _tile_lhs_pairs(w, nm, nkt):
    """[K, M] -> [nm/2, 128, 2, nkt, 128]; [mb][p][j][kt][n] = w[kt*128+p, (2mb+j)*128+n]."""
    K, M = w.shape
    assert K == nkt * 128 and M == nm * 128 and nm % 2 == 0
    return np.ascontiguousarray(
        w.reshape(nkt, 128, nm // 2, 2, 128).transpose(2, 1, 3, 0, 4))


def _tile_lhs(w, nm, nkt):
    """[K, M] -> [nm, 128, nkt, 128]."""
    K, M = w.shape
    assert K == nkt * 128 and M == nm * 128
    return np.ascontiguousarray(
        w.reshape(nkt, 128, nm, 128).transpose(2, 1, 0, 3))


def _tile_ff1_pairs(w):
    """[1024, 8192] -> [32, 128, 2, 8, 128]; [i][p][0]=gate tile 32+i, [i][p][1]=a tile i."""
    t = _tile_lhs(w, 64, 8)           # [64, 128, 8, 128]
    return np.ascontiguousarray(
        np.stack([t[32:64], t[0:32]], axis=2))


def _tile_rhs(w, nkt):
    K, N = w.shape
    assert K == nkt * 128 and N % 512 == 0
    return np.ascontiguousarray(
        w.reshape(nkt, 128, N // 512, 512).transpose(2, 1, 0, 3))


def _bias_cols(b, ncols):
    return np.ascontiguousarray(np.asarray(b, np.float32).reshape(ncols, 128).T)


_NC_CACHE = None


def kernel(**inputs):
    global _NC_CACHE
    inp = {k: np.asarray(v, np.float32) for k, v in inputs.items()}

    x = inp["x"][0]                    # [T, DIM]
    ctx = inp["context"][0]            # [77, CTX]
    xT_full = np.ascontiguousarray(x.T)
    ctxT = np.zeros((CTX, TCXP), np.float32)
    ctxT[:, :TCX] = ctx.T

    wq1 = np.ascontiguousarray((inp["n1_w"][:, None] * inp["q1_w"]) * SCALE)
    wk1 = np.ascontiguousarray(inp["n1_w"][:, None] * inp["k1_w"])
    wv1 = np.ascontiguousarray(inp["n1_w"][:, None] * inp["v1_w"])
    qb1 = (inp["n1_b"] @ inp["q1_w"]) * SCALE
    kb1 = inp["n1_b"] @ inp["k1_w"]
    vb1 = inp["n1_b"] @ inp["v1_w"]
    wq2 = np.ascontiguousarray((inp["n2_w"][:, None] * inp["q2_w"]) * SCALE)
    qb2 = (inp["n2_b"] @ inp["q2_w"]) * SCALE
    ff1 = np.ascontiguousarray(inp["n3_w"][:, None] * inp["ff1_w"])
    fb1 = inp["n3_b"] @ inp["ff1_w"] + inp["ff1_b"]

    sel16 = np.zeros((16, 8, 128), np.float32)
    for p in range(PAIRS):
        sel16[2 * p, p, 0:64] = 1.0
        sel16[2 * p + 1, p, 64:128] = 1.0

    shared = {
        "ctxT": ctxT,
        "sel16": sel16,
        "wq1t": _tile_lhs_pairs(wq1, 8, CKT),
        "wk1t": _tile_lhs_pairs(wk1, 8, CKT),
        "wv1t": _tile_rhs(wv1, CKT),
        "o1t": _tile_lhs_pairs(np.ascontiguousarray(inp["o1_w"]), 8, CKT),
        "wq2t": _tile_lhs_pairs(wq2, 8, CKT),
        "k2t": _tile_lhs_pairs(np.ascontiguousarray(inp["k2_w"]), 8, CKT_CTX),
        "v2t": _tile_rhs(np.ascontiguousarray(inp["v2_w"]), CKT_CTX),
        "o2t": _tile_lhs_pairs(np.ascontiguousarray(inp["o2_w"]), 8, CKT),
        "ff1t": _tile_ff1_pairs(ff1),
        "ff2t": _tile_lhs(np.ascontiguousarray(inp["ff2_w"]), 8, FF // 128),
        "vb1r": np.ascontiguousarray(vb1.reshape(1, DIM)),
    }
    f32_shared = {
        "qb1c": _bias_cols(qb1, 8),
        "kb1c": _bias_cols(kb1, 8),
        "o1bc": _bias_cols(inp["o1_b"], 8),
        "qb2c": _bias_cols(qb2, 8),
        "o2bc": _bias_cols(inp["o2_b"], 8),
        "fb1c": _bias_cols(fb1, 64),
        "ff2bc": _bias_cols(inp["ff2_b"], 8),
        "padmask": np.ascontiguousarray(
            (np.arange(128)[:, None] < TCX).astype(np.float32)
            * np.ones((1, 16), np.float32)),
    }
    shared = {k: np.ascontiguousarray(v, dtype=ml_dtypes.bfloat16)
              for k, v in shared.items()}
    shared.update({k: np.ascontiguousarray(v, dtype=np.float32)
                   for k, v in f32_shared.items()})

    in_maps = []
    for c in range(NCORES):
        m = dict(shared)
        m["xT"] = np.ascontiguousarray(xT_full[:, c * TO:(c + 1) * TO])
        m["rank_in"] = np.full((1, 1), float(c), np.float32)
        in_maps.append(m)

    if _NC_CACHE is None:
        _NC_CACHE = build_nc()
    nc = _NC_CACHE

    res = run_bass_kernel_spmd(nc, in_maps, core_ids=list(range(NCORES)))

    outs = [res.results[c]["outT"].T for c in range(NCORES)]   # each [TO, DIM]
    return np.ascontiguousarray(np.concatenate(outs, axis=0))[None].astype(np.float32)


if __name__ == "__main__":
    d = np.load("/tmp/ref_inputs.npz")
    out = kernel(**{k: d[k] for k in d.files})
    ref = np.load("/tmp/ref_out.npy")
    err = np.abs(out - ref).max()
    print("max abs err:", err, " absmax ref:", np.abs(ref).max(),
          " rel:", err / np.abs(ref).max())


# revision 28
# speedup vs baseline: 1.0888x; 1.0888x over previous
"""Trainium2 Bass kernel for nn_BasicTransformerBlock (self-attn + cross-attn
+ GEGLU FF, dim=1024, heads=16, seq=4096, ctx=77).

Strategy (8 NeuronCores), v2:
 - Sequence-parallel: each core owns 512 tokens end-to-end, activations kept
   transposed on-chip ([channel, token]).
 - K and augmented-V for self-attention are fp8 (e4m3); one fused AllGather
   ships both (half the wire bytes of the bf16 baseline). The AG bubble is
   filled with the Q projection, the cross-attn K2/V2 projections, and the
   "diagonal" score+exp prefill (own-rank key tiles from SBUF).
 - Softmax without max-subtraction. exp is split across ScalarE (table exp,
   fp8 out) and VectorE (integer Schraudolph: bits = round(s*8/ln2 + 8*(7-C))
   written as uint8, bit-identical to fp8e4m3) - the two engines each handle
   ~half of the 33M exps, which was the baseline's critical path.
 - P^T and V both fp8 => AV matmuls run in DoubleRow mode (256-deep virtual
   contraction), halving the attention AV matmul count. The V rows carry a
   trailing ones column per head so softmax denominators fall out of the AV
   matmul for free.
 - All row broadcasts (LN scale/shift, softmax 1/z) are K=1/K=2 matmuls on
   TensorE instead of DRAM-bounce broadcast DMAs.
 - Weights stay bf16 (fp8 weights push rel-err to ~2e-2). Weight DMAs are
   batched in 2-m-tile pairs and issued on the scalar HWDGE queue so they
   never block attention-critical loads on the sync queue; FF1/FF2 tiles are
   prefetched during attention.
Host gathers the 8 transposed output shards and transposes back.
"""
import numpy as np
import ml_dtypes
from contextlib import ExitStack

import concourse.bass as bass
import concourse.tile as tile
import concourse.mybir as mybir
from concourse.bass_utils import run_bass_kernel_spmd


# --- inlined BIR sync-wait legalizer (toolchain accepts max 1 wait/inst) ---
import json as _json


def _legalize_bir_json(raw, max_waits=1):
    d = _json.loads(raw)
    ctr = 0
    for f in d.get("functions", []):
        for bb in f.get("blocks", []):
            out = []
            for ins in bb.get("instructions", []):
                si = ins.get("sync_info")
                if si:
                    waits = si.get("on_wait") or []
                    if len(waits) > max_waits:
                        extra, keep = waits[:-max_waits], waits[-max_waits:]
                        for w in extra:
                            ctr += 1
                            out.append({
                                "debug": ins.get("debug", 0),
                                "engine": ins["engine"],
                                "ins": [],
                                "outs": [],
                                "name": f"waitfix-{ctr}",
                                "opcode": "EventSemaphore",
                                "sync_info": {"on_update": [], "on_wait": [w]},
                            })
                        si["on_wait"] = keep
                    ups = si.get("on_update") or []
                    if len(ups) > 1:
                        raise AssertionError(
                            f"instruction {ins.get('name')} has {len(ups)} updates")
                out.append(ins)
            bb["instructions"] = out
    return _json.dumps(d).encode()


def _install_legalizer(max_waits=1):
    import concourse.bass as _bassmod

    if getattr(_bassmod.Bass, "_legalize_installed", False):
        return
    orig = _bassmod.Bass.to_json_bytes

    def patched(self):
        return _legalize_bir_json(orig(self), max_waits=max_waits)

    _bassmod.Bass.to_json_bytes = patched
    _bassmod.Bass._legalize_installed = True


_install_legalizer()


def _install_ldwopt():
    import concourse.bass_utils as _bu

    if getattr(_bu, "_ldwopt_patched", False):
        return
    _orig_rc = _bu.run_command

    def _rc(cmd, **kw):
        try:
            cmd = ["--enable-ldw-opt=true" if c == "--enable-ldw-opt=false"
                   else c for c in cmd]
        except TypeError:
            pass
        return _orig_rc(cmd, **kw)

    _bu.run_command = _rc
    _bu._ldwopt_patched = True


# NOTE: --enable-ldw-opt=true rejects DoubleRow InstLdweights; left disabled.
# _install_ldwopt()

F32 = mybir.dt.float32
F32R = mybir.dt.float32r
BF16 = mybir.dt.bfloat16
F8 = mybir.dt.float8e4
U8 = mybir.dt.uint8
AF = mybir.ActivationFunctionType
OP = mybir.AluOpType
DR = mybir.MatmulPerfMode.DoubleRow

DIM = 1024
HEADS = 16
D = 64
CTX = 768
FF = 4096
T = 4096
NCORES = 8
TO = T // NCORES          # 512 own tokens per core
PAIRS = HEADS // 2        # 8 head pairs
CKT = DIM // 128          # 8 contraction tiles over DIM
CKT_CTX = CTX // 128      # 6 contraction tiles over CTX
TCX = 77
TCXP = 80                 # ctx tokens padded
SCALE = D ** -0.5
EPS = 1e-5

# V augmented row layout (fp8): per pair a 160-col block:
#   [0:64)  V head A   [64] ones A   [65:80) pad
#   [80:144) V head B  [144] ones B  [145:160) pad
PB = 160                     # pair block width
V_ROWP = PAIRS * PB          # 1280
K_ELEMS = DIM * TO           # 524288 fp8 bytes
V_ELEMS = TO * V_ROWP        # 655360
AG_ELEMS = K_ELEMS + V_ELEMS

# Schraudolph fp8 exp constants (validated bit-exact vs HW probe)
LOG2E = 1.4426950408889634
SCH_C = 0.0430
SCH_A8 = 8.0 * LOG2E
SCH_B8 = 8.0 * (7.0 - SCH_C)


def _ap(tensor_ap, offset, steps):
    """Raw AP view on a (flat) dram tensor: steps = [[step, count], ...]."""
    return bass.AP(tensor=tensor_ap.tensor, offset=tensor_ap.offset + offset,
                   ap=list(steps))


def build_nc(fake_ag=False):
    nc = bass.Bass(trn_type="TRN2")

    # ---- dram tensors ----------------------------------------------------
    xT = nc.dram_tensor("xT", [DIM, TO], F32, kind="ExternalInput")
    ctxT = nc.dram_tensor("ctxT", [CTX, TCXP], BF16, kind="ExternalInput")

    def w_in(name, shape=None, dt=BF16, shape_=None):
        return nc.dram_tensor(name, list(shape if shape is not None else shape_),
                              dt, kind="ExternalInput")

    # paired m-tile layouts: [nm/2, 128, 2, nkt, 128]
    wq1t = w_in("wq1t", (4, 128, 2, CKT, 128))
    wk1t = w_in("wk1t", (4, 128, 2, CKT, 128))
    wv1t = w_in("wv1t", (2, 128, CKT, 512))
    o1t = w_in("o1t", (4, 128, 2, CKT, 128))
    wq2t = w_in("wq2t", (4, 128, 2, CKT, 128))
    k2t = w_in("k2t", (4, 128, 2, CKT_CTX, 128))
    v2t = w_in("v2t", (2, 128, CKT_CTX, 512))
    o2t = w_in("o2t", (4, 128, 2, CKT, 128))
    ff1t = w_in("ff1t", (32, 128, 2, CKT, 128))   # [i] = (gate 32+i, a i)
    ff2t = w_in("ff2t", (8, 128, FF // 128, 128))

    sel16d = w_in("sel16", dt=BF16, shape_=(16, 8, 128))
    qb1c = w_in("qb1c", dt=F32, shape_=(128, 8))
    kb1c = w_in("kb1c", dt=F32, shape_=(128, 8))
    vb1r = w_in("vb1r", dt=BF16, shape_=(1, DIM))
    o1bc = w_in("o1bc", dt=F32, shape_=(128, 8))
    qb2c = w_in("qb2c", dt=F32, shape_=(128, 8))
    o2bc = w_in("o2bc", dt=F32, shape_=(128, 8))
    fb1c = w_in("fb1c", dt=F32, shape_=(128, 64))
    padmask = w_in("padmask", dt=F32, shape_=(128, 16))
    ff2bc = w_in("ff2bc", dt=F32, shape_=(128, 8))

    outT = nc.dram_tensor("outT", [DIM, TO], F32, kind="ExternalOutput")

    with tile.TileContext(nc) as tc, ExitStack() as top:
        dram = top.enter_context(tc.tile_pool(name="dram", bufs=1, space="DRAM"))
        p_const = top.enter_context(tc.tile_pool(name="p_const", bufs=1))

        # ---- constants ---------------------------------------------------
        ones_col_f = p_const.tile([128, 1], F32, name="ones_col_f")
        nc.vector.memset(ones_col_f[:], 1.0)
        ones_col = p_const.tile([128, 1], F32R, name="ones_col")
        nc.scalar.copy(ones_col[:], ones_col_f[:])
        ones_row_bf = p_const.tile([1, 128], BF16, name="ones_row_bf")
        nc.vector.memset(ones_row_bf[:], 1.0)
        # sel16[:, p, :]: K=16 selector that broadcasts 1/z rows 2p (to
        # partitions 0:64) and 2p+1 (to 64:128)
        sel16 = p_const.tile([16, 8, 128], BF16, name="sel16")
        nc.sync.dma_start(out=sel16, in_=sel16d.ap())
        padones = p_const.tile([128, 16], F32, name="padones")
        nc.sync.dma_start(out=padones, in_=padmask.ap())
        eps_row = p_const.tile([1, 1], F32, name="eps_row")
        nc.vector.memset(eps_row[:], EPS)

        def bias_tile(name, dram_t, cols):
            t = p_const.tile([128, cols], F32, name=name)
            nc.sync.dma_start(out=t, in_=dram_t.ap())
            return t

        qb1 = bias_tile("qb1", qb1c, 8)
        kb1 = bias_tile("kb1", kb1c, 8)
        o1b = bias_tile("o1b", o1bc, 8)
        qb2 = bias_tile("qb2", qb2c, 8)
        o2b = bias_tile("o2b", o2bc, 8)
        fb1 = bias_tile("fb1", fb1c, 64)
        ff2b = bias_tile("ff2b", ff2bc, 8)
        # vb1 broadcast via K=1 matmul at V-evac time needs [128, DIM] view;
        # build it once into SBUF from a psum broadcast.
        vb1row = p_const.tile([1, DIM], BF16, name="vb1row")
        nc.sync.dma_start(out=vb1row, in_=vb1r.ap())
        vb1bc = p_const.tile([128, DIM], F32, name="vb1bc")
        with ExitStack() as st0:
            psb0 = st0.enter_context(tc.tile_pool(name="psb0", bufs=1, space="PSUM"))
            pv = psb0.tile([128, 512], F32, name="pv", tag="pv")
            for half in range(2):
                nc.tensor.matmul(pv[:], ones_row_bf[:],
                                 vb1row[:, half * 512:(half + 1) * 512],
                                 start=True, stop=True)
                nc.vector.tensor_copy(vb1bc[:, half * 512:(half + 1) * 512], pv[:])
        ctx_sb = []
        for i in range(CKT_CTX):
            t = p_const.tile([128, TCXP], BF16, name=f"ctxsb{i}")
            nc.sync.dma_start(out=t, in_=ctxT.ap()[i * 128:(i + 1) * 128, :])
            ctx_sb.append(t)

        # exp engine alternation counter
        exp_state = {"i": 0}

        def exp_to_fp8(out_ap, in_ap):
            """exp of a PSUM tile into an fp8 SBUF AP; alternates ACT/DVE."""
            if exp_state["i"] % 2 == 0:
                nc.scalar.activation(out_ap, in_ap, AF.Exp)
            else:
                nc.vector.tensor_scalar(out_ap.bitcast(U8), in_ap,
                                        SCH_A8, SCH_B8, op0=OP.mult, op1=OP.add)
            exp_state["i"] += 1

        def exp_split(out4, lt, pss):
            """exp of one [128,2,TO] score tile: head A on ScalarE (table),
            head B on VectorE (integer Schraudolph). Half the latency of a
            single-engine pass, so score PSUM frees faster."""
            nc.scalar.activation(out4[:, lt, 0, :], pss[:, 0, :], AF.Exp)
            nc.vector.tensor_scalar(out4[:, lt, 1, :].bitcast(U8), pss[:, 1, :],
                                    SCH_A8, SCH_B8, op0=OP.mult, op1=OP.add)

        def exp_to_bf16(out_ap, in_ap):
            if exp_state["i"] % 2 == 0:
                nc.scalar.activation(out_ap, in_ap, AF.Exp)
            else:
                nc.vector.tensor_scalar(out_ap.bitcast(mybir.dt.uint16), in_ap,
                                        128.0 * LOG2E, 128.0 * (127.0 - SCH_C),
                                        op0=OP.mult, op1=OP.add)
            exp_state["i"] += 1

        # ---- helpers -----------------------------------------------------
        def layernorm(xtiles, h_pool, tag, out_dtype=BF16):
            """xtiles: 8 sbuf tiles [128, TO] F32R -> 8 out tiles [128,TO]."""
            with ExitStack() as ln:
                work = ln.enter_context(tc.tile_pool(name=f"lnw_{tag}", bufs=2))
                rows = ln.enter_context(tc.tile_pool(name=f"lnr_{tag}", bufs=1))
                ps = ln.enter_context(tc.tile_pool(name=f"lnp_{tag}", bufs=1,
                                                   space="PSUM"))
                ps_s = ps.tile([1, TO], F32, name=f"pss_{tag}", tag="s")
                ps_q = ps.tile([1, TO], F32, name=f"psq_{tag}", tag="q")
                for i in range(8):
                    sq = work.tile([128, TO], F32R, name=f"sq_{tag}", tag="sq")
                    nc.gpsimd.tensor_tensor(sq[:], xtiles[i].bitcast(F32),
                                            xtiles[i].bitcast(F32), op=OP.mult)
                    nc.tensor.matmul(ps_s[:], ones_col[:], xtiles[i][:],
                                     start=(i == 0), stop=(i == 7))
                    nc.tensor.matmul(ps_q[:], ones_col[:], sq[:],
                                     start=(i == 0), stop=(i == 7))
                mu = rows.tile([1, TO], F32, name=f"mu_{tag}")
                nc.vector.tensor_scalar(mu[:], ps_s[:], 1.0 / DIM, None, op0=OP.mult)
                m2 = rows.tile([1, TO], F32, name=f"m2_{tag}")
                nc.vector.tensor_scalar(m2[:], ps_q[:], 1.0 / DIM, None, op0=OP.mult)
                var = rows.tile([1, TO], F32, name=f"var_{tag}")
                nc.vector.tensor_tensor(var[:], mu[:], mu[:], op=OP.mult)
                nc.vector.tensor_tensor(var[:], m2[:], var[:], op=OP.subtract)
                # 1/sqrt(v+eps) = exp(-0.5*ln(v+eps)); table ops beat the
                # 3.3us single-row DVE reciprocal
                lnv = rows.tile([1, TO], F32, name=f"lnv_{tag}")
                nc.scalar.activation(lnv[:], var[:], AF.Ln, bias=eps_row[:])
                ra = rows.tile([1, TO], F32, name=f"ra_{tag}")
                nc.scalar.activation(ra[:], lnv[:], AF.Exp, scale=-0.5)
                rb = rows.tile([1, TO], F32, name=f"rb_{tag}")
                nc.vector.scalar_tensor_tensor(rb[:], mu[:], -1.0, ra[:],
                                               op0=OP.mult, op1=OP.mult)
                rab = rows.tile([1, TO], BF16, name=f"rab_{tag}")
                nc.scalar.copy(rab[:], ra[:])
                rbb = rows.tile([1, TO], BF16, name=f"rbb_{tag}")
                nc.scalar.copy(rbb[:], rb[:])
                ps_a = ps.tile([128, TO], F32, name=f"psa_{tag}", tag="a")
                ps_b = ps.tile([128, TO], F32, name=f"psb_{tag}", tag="b")
                nc.tensor.matmul(ps_a[:], ones_row_bf[:], rab[:],
                                 start=True, stop=True)
                nc.tensor.matmul(ps_b[:], ones_row_bf[:], rbb[:],
                                 start=True, stop=True)
                bsb = rows.tile([128, TO], F32, name=f"bsb_{tag}")
                nc.vector.tensor_copy(bsb[:], ps_b[:])
                out = []
                for i in range(8):
                    tmp = work.tile([128, TO], F32, name=f"tmp_{tag}", tag="tmp")
                    nc.vector.tensor_tensor(tmp[:], xtiles[i].bitcast(F32),
                                            ps_a[:], op=OP.mult)
                    h = h_pool.tile([128, TO], out_dtype, name=f"h_{tag}{i}")
                    nc.gpsimd.tensor_tensor(h[:], tmp[:], bsb[:], op=OP.add)
                    out.append(h)
                return out

        def proj_T(wdram, rhs_tiles, bias, out_pool, tag, nkt=CKT,
                   out_dtype=BF16, residual=None, res_bias=None, nmb=4,
                   store_dram=None):
            """Paired m-tile projection. wdram: [nmb, 128, 2, nkt, 128]."""
            outs = []
            with ExitStack() as st:
                wp = st.enter_context(tc.tile_pool(name=f"wp_{tag}", bufs=2))
                ps = st.enter_context(tc.tile_pool(name=f"ps_{tag}", bufs=2,
                                                   space="PSUM"))
                wtiles = []
                for mb in range(min(2, nmb)):
                    wm = wp.tile([128, 2, nkt, 128], BF16, name=f"wm_{tag}", tag="w")
                    nc.sync.dma_start(out=wm, in_=wdram.ap()[mb])
                    wtiles.append(wm)
                for mb in range(nmb):
                    wm = wtiles[mb]
                    for j in range(2):
                        m = 2 * mb + j
                        psy = ps.tile([128, TO], F32, name=f"psy_{tag}", tag="y")
                        for kt in range(nkt):
                            nc.tensor.matmul(psy[:], wm[:, j, kt, :],
                                             rhs_tiles[kt][:],
                                             start=(kt == 0), stop=(kt == nkt - 1))
                        o = out_pool.tile([128, TO], out_dtype, name=f"o_{tag}{m}")
                        if residual is not None:
                            nc.vector.scalar_tensor_tensor(
                                o[:], psy[:], res_bias[:, m:m + 1],
                                residual[m].bitcast(F32), op0=OP.add, op1=OP.add)
                        elif bias is not None:
                            nc.vector.tensor_scalar(o[:], psy[:], bias[:, m:m + 1],
                                                    None, op0=OP.add)
                        else:
                            nc.vector.tensor_copy(o[:], psy[:])
                        if store_dram is not None:
                            nc.sync.dma_start(
                                out=_ap(store_dram, m * 128 * TO,
                                        [[TO, 128], [1, TO]]),
                                in_=o[:])
                        outs.append(o)
                    if mb + 2 < nmb:
                        wn = wp.tile([128, 2, nkt, 128], BF16, name=f"wm_{tag}",
                                     tag="w")
                        nc.sync.dma_start(out=wn, in_=wdram.ap()[mb + 2])
                        wtiles.append(wn)
            return outs

        # ---- AG buffer (fused K + augmented V, fp8) ----------------------
        agkv_in = dram.tile([AG_ELEMS], F8, name="agkv_in")
        agkv_out = dram.tile([NCORES * AG_ELEMS], F8, name="agkv_out",
                             addr_space="Local" if fake_ag else "Shared")

        # ================= phase A: LN1 + K/V/Q projections ===============
        p_xT = top.enter_context(tc.tile_pool(name="p_xT", bufs=1))
        p_QT = top.enter_context(tc.tile_pool(name="p_QT", bufs=1))
        p_OT = top.enter_context(tc.tile_pool(name="p_OT", bufs=1))
        p_kv = top.enter_context(tc.tile_pool(name="p_kv", bufs=1))

        xtiles = []
        for i in range(8):
            t = p_xT.tile([128, TO], F32R, name=f"xT{i}")
            nc.sync.dma_start(out=t,
                              in_=xT.ap()[i * 128:(i + 1) * 128, :].bitcast(F32R))
            xtiles.append(t)

        with ExitStack() as phA:
            p_h1 = phA.enter_context(tc.tile_pool(name="p_h1", bufs=1))
            h1 = layernorm(xtiles, p_h1, "ln1")

            # K^T own (fp8) -> SBUF (for diagonal) + agkv_in rows [0 : DIM)
            ko = proj_T(wk1t, h1, kb1, p_kv, "k1", out_dtype=F8,
                        store_dram=agkv_in[:])

            # V own augmented (fp8) -> vag4 [128, 4, V_ROWP] + agkv_in
            vag4 = p_kv.tile([128, 4, PAIRS, PB], F8, name="vag4")
            with ExitStack() as stv:
                wvp = stv.enter_context(tc.tile_pool(name="wp_v1", bufs=1))
                ps = stv.enter_context(tc.tile_pool(name="ps_v1", bufs=2,
                                                    space="PSUM"))
                wv_sb = []
                for nb in range(2):
                    w = wvp.tile([128, CKT, 512], BF16, name=f"wv{nb}")
                    nc.sync.dma_start(out=w, in_=wv1t.ap()[nb])
                    wv_sb.append(w)
                for t4 in range(4):
                    for nb in range(2):
                        psv = ps.tile([128, 512], F32, name="psv", tag="v")
                        for kt in range(CKT):
                            nc.tensor.matmul(
                                psv[:], h1[kt][:, t4 * 128:(t4 + 1) * 128],
                                wv_sb[nb][:, kt, :],
                                start=(kt == 0), stop=(kt == CKT - 1))
                        # psv inner layout: 8 heads x 64; heads hh=0..7 map to
                        # pair pp = 4*nb + hh//2, half b = hh%2 at col 80b..80b+64
                        psv3 = psv.rearrange("p (g two e) -> p g two e", two=2, e=D)
                        vslice = vag4[:, t4, 4 * nb:4 * nb + 4, :]
                        for b in range(2):
                            nc.vector.tensor_tensor(
                                vslice[:, :, 80 * b:80 * b + D],
                                psv3[:, :, b, :],
                                vb1bc[:, nb * 512:(nb + 1) * 512].rearrange(
                                    "p (g two e) -> p g two e", two=2, e=D)[:, :, b, :],
                                op=OP.add)
                # ones columns (denominator rows) for both halves of all pairs
                for b in range(2):
                    nc.vector.memset(vag4[:, :, :, 80 * b + D:80 * b + D + 1], 1.0)
                nc.sync.dma_start(
                    out=_ap(agkv_in[:], K_ELEMS,
                            [[V_ROWP, 128], [128 * V_ROWP, 4], [1, V_ROWP]]),
                    in_=vag4[:])

            if fake_ag:
                for r in range(NCORES):
                    nc.sync.dma_start(
                        out=_ap(agkv_out[:], r * AG_ELEMS, [[1, AG_ELEMS]]),
                        in_=_ap(agkv_in[:], 0, [[1, AG_ELEMS]]))
            else:
                nc.gpsimd.collective_compute(
                    "AllGather", OP.bypass,
                    replica_groups=[list(range(NCORES))],
                    ins=[agkv_in[:]], outs=[agkv_out[:]])

            QT = proj_T(wq1t, h1, qb1, p_QT, "q1", out_dtype=F8)

        # ---- cross-attn K2/V2 from context (fills the AG bubble) ---------
        p_kv2 = top.enter_context(tc.tile_pool(name="p_kv2", bufs=1))
        K2T = []
        with ExitStack() as stk2:
            wp = stk2.enter_context(tc.tile_pool(name="wp_k2", bufs=2))
            ps = stk2.enter_context(tc.tile_pool(name="ps_k2", bufs=2, space="PSUM"))
            for mb in range(4):
                wm = wp.tile([128, 2, CKT_CTX, 128], BF16, name="wm_k2", tag="w")
                nc.sync.dma_start(out=wm, in_=k2t.ap()[mb])
                for j in range(2):
                    psy = ps.tile([128, TCXP], F32, name="psy_k2", tag="y")
                    for kt in range(CKT_CTX):
                        nc.tensor.matmul(psy[:], wm[:, j, kt, :], ctx_sb[kt][:],
                                         start=(kt == 0), stop=(kt == CKT_CTX - 1))
                    k2 = p_kv2.tile([128, TCXP], BF16, name=f"k2_{2 * mb + j}")
                    nc.vector.tensor_copy(k2[:], psy[:])
                    K2T.append(k2)

        V2_ROW = HEADS * (D + 1)  # 1040, ones at end of each 65-block
        v2ag = p_kv2.tile([TCXP, V2_ROW], BF16, name="v2ag")
        v2ag3 = v2ag.rearrange("p (h e) -> p h e", e=D + 1)
        with ExitStack() as stv2:
            wvp = stv2.enter_context(tc.tile_pool(name="wp_v2", bufs=1))
            ps = stv2.enter_context(tc.tile_pool(name="ps_v2", bufs=2, space="PSUM"))
            for nb in range(2):
                w = wvp.tile([128, CKT_CTX, 512], BF16, name=f"wv2_{nb}", tag="w")
                nc.sync.dma_start(out=w, in_=v2t.ap()[nb])
                psv = ps.tile([TCXP, 512], F32, name="psv2", tag="v")
                for kt in range(CKT_CTX):
                    nc.tensor.matmul(psv[:], ctx_sb[kt][:], w[:, kt, :],
                                     start=(kt == 0), stop=(kt == CKT_CTX - 1))
                nc.vector.tensor_copy(
                    v2ag3[:, nb * 8:(nb + 1) * 8, 0:D],
                    psv[:].rearrange("p (h e) -> p h e", e=D))
            nc.scalar.copy(v2ag3[:, :, D:D + 1], padones[0:TCXP, :].unsqueeze(2))

        # ================= phase B: self-attention ========================
        p_ff1w = top.enter_context(tc.tile_pool(name="p_ff1w", bufs=8))
        p_ff2w = top.enter_context(tc.tile_pool(name="p_ff2w", bufs=2))

        with ExitStack() as phB:
            ps_S = phB.enter_context(tc.tile_pool(name="ps_S", bufs=2, space="PSUM"))
            ps_AV = phB.enter_context(tc.tile_pool(name="ps_AV", bufs=1,
                                                   space="PSUM"))
            ps_BC = phB.enter_context(tc.tile_pool(name="ps_BC", bufs=2,
                                                   space="PSUM"))
            p_at = phB.enter_context(tc.tile_pool(name="p_at", bufs=2))
            p_pt = phB.enter_context(tc.tile_pool(name="p_pt", bufs=2))
            p_rb = phB.enter_context(tc.tile_pool(name="p_rb", bufs=2))

            def score_step(pss, lhsA, lhsB, qt):
                nc.tensor.matmul(pss[:, 0, :], lhsA, qt[0:64, :],
                                 start=True, stop=True, tile_position=(0, 0))
                nc.tensor.matmul(pss[:, 1, :], lhsB, qt[64:128, :],
                                 start=True, stop=True, tile_position=(64, 0))

            # --- bubble fill: own-token scores+exp warm-up from SBUF ------
            # (SPMD program cannot branch on rank, so these exps are
            # recomputed from the gathered buffer below; this pass just keeps
            # PE/ACT/DVE warm while the AllGather is in flight.)
            for p in range(PAIRS):
                pt = p_pt.tile([128, 4, 2, TO], F8, name="ptm", tag="pt")
                for lt in range(4):
                    pss = ps_S.tile([128, 2, TO], F32, name="pss", tag="s")
                    score_step(pss,
                               ko[p][0:64, lt * 128:(lt + 1) * 128],
                               ko[p][64:128, lt * 128:(lt + 1) * 128],
                               QT[p])
                    exp_to_fp8(pt[:, lt, :, :], pss[:])

            # --- prefetch first FF1 weight pairs on the scalar queue ------
            ff1_tiles = [None] * 32
            for i in range(8):
                wt = p_ff1w.tile([128, 2, CKT, 128], BF16, name="wff1", tag="w")
                nc.scalar.dma_start(out=wt, in_=ff1t.ap()[i])
                ff1_tiles[i] = wt

            # --- main loop ------------------------------------------------
            zsb = p_rb.tile([16, TO], F32, name="zsb", bufs=1)
            OT = []
            def load_pair(p):
                kpair = p_at.tile([128, NCORES, TO], F8, name="kpair", tag="kp")
                nc.gpsimd.dma_start(
                    out=kpair[:],
                    in_=_ap(agkv_out[:], p * 128 * TO,
                            [[TO, 128], [AG_ELEMS, NCORES], [1, TO]]))
                vpr = p_at.tile([128, NCORES, 4, PB], F8, name="vpr", tag="vp")
                for r in range(NCORES):
                    nc.gpsimd.dma_start(
                        out=vpr[:, r, :, :],
                        in_=_ap(agkv_out[:], r * AG_ELEMS + K_ELEMS + p * PB,
                                [[V_ROWP, 128], [128 * V_ROWP, 4], [1, PB]]))
                return kpair, vpr

            loaded = {0: load_pair(0)}
            for p in range(PAIRS):
                kpair, vpr = loaded.pop(p)
                if p + 1 < PAIRS:
                    loaded[p + 1] = load_pair(p + 1)

                psA = ps_AV.tile([65, 2, TO], F32, name="psA", tag="A")

                def av_step(r, ptm):
                    for i in range(2):
                        nc.tensor.matmul(
                            psA[:, 0, :],
                            vpr[:, r, 2 * i:2 * i + 2, 0:D + 1],
                            ptm[:, 2 * i:2 * i + 2, 0, :],
                            start=(r == 0 and i == 0),
                            stop=(r == NCORES - 1 and i == 1),
                            perf_mode=DR)
                        nc.tensor.matmul(
                            psA[:, 1, :],
                            vpr[:, r, 2 * i:2 * i + 2, 80:80 + D + 1],
                            ptm[:, 2 * i:2 * i + 2, 1, :],
                            start=(r == 0 and i == 0),
                            stop=(r == NCORES - 1 and i == 1),
                            perf_mode=DR)

                # software pipeline: scores+exp for rank r while AV of r-1
                # runs, so the in-order PE queue never stalls on exp results
                ptm_prev = None
                for r in range(NCORES):
                    ptm = p_pt.tile([128, 4, 2, TO], F8, name="ptm", tag="pt")
                    for lt in range(4):
                        pss = ps_S.tile([128, 2, TO], F32, name="pss", tag="s")
                        score_step(pss,
                                   kpair[0:64, r, lt * 128:(lt + 1) * 128],
                                   kpair[64:128, r, lt * 128:(lt + 1) * 128],
                                   QT[p])
                        exp_to_fp8(ptm[:, lt, :, :], pss[:])
                    if ptm_prev is not None:
                        av_step(r - 1, ptm_prev)
                    ptm_prev = ptm
                av_step(NCORES - 1, ptm_prev)

                # stash unnormalized AV + z rows; batched normalize below
                zrA = p_rb.tile([1, TO], F32, name="zrA", tag="zra")
                nc.vector.tensor_copy(zrA[:], psA[D:D + 1, 0, :])
                nc.gpsimd.dma_start(out=zsb[2 * p:2 * p + 1, :], in_=zrA[:])
                zrB = p_rb.tile([1, TO], F32, name="zrB", tag="zrb")
                nc.vector.tensor_copy(zrB[:], psA[D:D + 1, 1, :])
                nc.gpsimd.dma_start(out=zsb[2 * p + 1:2 * p + 2, :], in_=zrB[:])
                ot = p_OT.tile([128, TO], BF16, name=f"ot{p}")
                nc.vector.tensor_copy(ot[0:64, :], psA[0:D, 0, :])
                nc.scalar.copy(ot[64:128, :], psA[0:D, 1, :])
                OT.append(ot)

            # batched softmax divide: one 16-lane reciprocal, then per-pair
            # K=16 selector matmul broadcast + in-place multiply
            zrec = p_rb.tile([16, TO], F32, name="zrec", bufs=1)
            nc.vector.reciprocal(zrec[:], zsb[:])
            zrecb = p_rb.tile([16, TO], BF16, name="zrecb", bufs=1)
            nc.scalar.copy(zrecb[:], zrec[:])
            for p in range(PAIRS):
                psbc = ps_BC.tile([128, TO], F32, name="psbc", tag="bc")
                nc.tensor.matmul(psbc[:], sel16[:, p, :], zrecb[:],
                                 start=True, stop=True)
                bcz = p_rb.tile([128, TO], BF16, name="bcz", tag="bcz")
                nc.vector.tensor_copy(bcz[:], psbc[:])
                nc.vector.tensor_tensor(OT[p][:], OT[p][:], bcz[:], op=OP.mult)

        # o1 projection + residual -> x2T
        p_x2 = top.enter_context(tc.tile_pool(name="p_x2", bufs=1))
        x2T = proj_T(o1t, OT, None, p_x2, "o1", residual=xtiles, res_bias=o1b,
                     out_dtype=F32R)

        # ================= phase C: cross-attention =======================
        p_x3 = top.enter_context(tc.tile_pool(name="p_x3", bufs=1))
        with ExitStack() as phC:
            p_Q2 = phC.enter_context(tc.tile_pool(name="p_Q2", bufs=1))
            p_OT2 = phC.enter_context(tc.tile_pool(name="p_OT2", bufs=1))

            with ExitStack() as stc:
                p_h2 = stc.enter_context(tc.tile_pool(name="p_h2", bufs=1))
                h2 = layernorm(x2T, p_h2, "ln2")
                Q2T = proj_T(wq2t, h2, qb2, p_Q2, "q2")

            with ExitStack() as stx:
                ps_S2 = stx.enter_context(tc.tile_pool(name="ps_S2", bufs=2,
                                                       space="PSUM"))
                ps_A2 = stx.enter_context(tc.tile_pool(name="ps_A2", bufs=1,
                                                       space="PSUM"))
                ps_B2 = stx.enter_context(tc.tile_pool(name="ps_B2", bufs=2,
                                                       space="PSUM"))
                p_pt2 = stx.enter_context(tc.tile_pool(name="p_pt2", bufs=2))
                p_rb2 = stx.enter_context(tc.tile_pool(name="p_rb2", bufs=2))
                zsb2 = p_rb2.tile([16, TO], F32, name="zsb2", bufs=1)
                OT2 = []
                for p in range(PAIRS):
                    pss = ps_S2.tile([TCXP, 2, TO], F32, name="pss2", tag="s")
                    nc.tensor.matmul(pss[:, 0, :], K2T[p][0:64, :], Q2T[p][0:64, :],
                                     start=True, stop=True, tile_position=(0, 0))
                    nc.tensor.matmul(pss[:, 1, :], K2T[p][64:128, :],
                                     Q2T[p][64:128, :],
                                     start=True, stop=True, tile_position=(64, 0))
                    pt = p_pt2.tile([TCXP, 2, TO], BF16, name="pt2", tag="pt")
                    exp_to_bf16(pt[:], pss[:])
                    psA = ps_A2.tile([D + 1, 2, TO], F32, name="psA2c", tag="A")
                    nc.tensor.matmul(psA[:, 0, :],
                                     v2ag[:, (2 * p) * (D + 1):(2 * p + 1) * (D + 1)],
                                     pt[:, 0, :], start=True, stop=True)
                    nc.tensor.matmul(psA[:, 1, :],
                                     v2ag[:, (2 * p + 1) * (D + 1):(2 * p + 2) * (D + 1)],
                                     pt[:, 1, :], start=True, stop=True)
                    zrA = p_rb2.tile([1, TO], F32, name="zrAc", tag="zra")
                    nc.vector.tensor_copy(zrA[:], psA[D:D + 1, 0, :])
                    nc.gpsimd.dma_start(out=zsb2[2 * p:2 * p + 1, :], in_=zrA[:])
                    zrB = p_rb2.tile([1, TO], F32, name="zrBc", tag="zrb")
                    nc.vector.tensor_copy(zrB[:], psA[D:D + 1, 1, :])
                    nc.gpsimd.dma_start(out=zsb2[2 * p + 1:2 * p + 2, :], in_=zrB[:])
                    ot = p_OT2.tile([128, TO], BF16, name=f"ot2_{p}")
                    nc.vector.tensor_copy(ot[0:64, :], psA[0:D, 0, :])
                    nc.scalar.copy(ot[64:128, :], psA[0:D, 1, :])
                    OT2.append(ot)

                zrec2 = p_rb2.tile([16, TO], F32, name="zrec2", bufs=1)
                nc.vector.reciprocal(zrec2[:], zsb2[:])
                zrecb2 = p_rb2.tile([16, TO], BF16, name="zrecb2", bufs=1)
                nc.scalar.copy(zrecb2[:], zrec2[:])
                for p in range(PAIRS):
                    psbc = ps_B2.tile([128, TO], F32, name="psbc2", tag="bc")
                    nc.tensor.matmul(psbc[:], sel16[:, p, :], zrecb2[:],
                                     start=True, stop=True)
                    bcz = p_rb2.tile([128, TO], BF16, name="bczc", tag="bcz")
                    nc.scalar.copy(bcz[:], psbc[:])
                    nc.vector.tensor_tensor(OT2[p][:], OT2[p][:], bcz[:],
                                            op=OP.mult)

            x3T = proj_T(o2t, OT2, None, p_x3, "o2", residual=x2T, res_bias=o2b,
                         out_dtype=F32R)

        # ================= phase D: GEGLU FF ==============================
        with ExitStack() as phD:
            p_hT = phD.enter_context(tc.tile_pool(name="p_hT", bufs=1))
            hT = []
            with ExitStack() as stf:
                p_h3 = stf.enter_context(tc.tile_pool(name="p_h3", bufs=1))
                h3 = layernorm(x3T, p_h3, "ln3")
                gp = stf.enter_context(tc.tile_pool(name="p_g", bufs=2))
                ps = stf.enter_context(tc.tile_pool(name="ps_ff1", bufs=3,
                                                    space="PSUM"))
                for i in range(32):
                    wt = ff1_tiles[i]
                    psg = ps.tile([128, TO], F32, name="psg", tag="p")
                    for kt in range(CKT):
                        nc.tensor.matmul(psg[:], wt[:, 0, kt, :], h3[kt][:],
                                         start=(kt == 0), stop=(kt == CKT - 1))
                    g = gp.tile([128, TO], F32, name="g", tag="g")
                    nc.scalar.activation(g[:], psg[:], AF.Gelu,
                                         bias=fb1[:, 32 + i:33 + i], scale=1.0)
                    psa = ps.tile([128, TO], F32, name="psa", tag="p")
                    for kt in range(CKT):
                        nc.tensor.matmul(psa[:], wt[:, 1, kt, :], h3[kt][:],
                                         start=(kt == 0), stop=(kt == CKT - 1))
                    h = p_hT.tile([128, TO], BF16, name=f"hT{i}")
                    nc.vector.scalar_tensor_tensor(h[:], psa[:], fb1[:, i:i + 1],
                                                   g[:], op0=OP.add, op1=OP.mult)
                    hT.append(h)
                    # stream in the next FF1 pair
                    if i + 8 < 32:
                        wt2 = p_ff1w.tile([128, 2, CKT, 128], BF16,
                                          name="wff1", tag="w")
                        nc.scalar.dma_start(out=wt2, in_=ff1t.ap()[i + 8])
                        ff1_tiles[i + 8] = wt2
                    if i == 0:
                        # kick off FF2 prefetch
                        ff2_tiles = []
                        for m in range(2):
                            w2 = p_ff2w.tile([128, FF // 128, 128], BF16,
                                             name="wff2", tag="w")
                            nc.scalar.dma_start(out=w2, in_=ff2t.ap()[m])
                            ff2_tiles.append(w2)

            with ExitStack() as stf2:
                outp = stf2.enter_context(tc.tile_pool(name="p_out", bufs=2))
                ps = stf2.enter_context(tc.tile_pool(name="ps_ff2", bufs=2,
                                                     space="PSUM"))
                for m in range(8):
                    wm = ff2_tiles[m % 2] if m < 2 else None
                    if m >= 2:
                        wm = p_ff2w.tile([128, FF // 128, 128], BF16,
                                         name="wff2", tag="w")
                        nc.scalar.dma_start(out=wm, in_=ff2t.ap()[m])
                    psy = ps.tile([128, TO], F32, name="psy_ff2", tag="y")
                    for kt in range(FF // 128):
                        nc.tensor.matmul(psy[:], wm[:, kt, :], hT[kt][:],
                                         start=(kt == 0), stop=(kt == FF // 128 - 1))
                    o = outp.tile([128, TO], F32, name="of", tag="of")
                    nc.vector.scalar_tensor_tensor(o[:], psy[:], ff2b[:, m:m + 1],
                                                   x3T[m].bitcast(F32),
                                                   op0=OP.add, op1=OP.add)
                    nc.sync.dma_start(out=outT.ap()[m * 128:(m + 1) * 128, :],
                                      in_=o[:])

    return nc


# ---------------------------------------------------------------------------
# host side
# ---------------------------------------------------------------------------
def _tile_lhs_pairs(w, nm, nkt):
    """[K, M] -> [nm/2, 128, 2, nkt, 128]; [mb][p][j][kt][n] = w[kt*128+p, (2mb+j)*128+n]."""
    K, M = w.shape
    assert K == nkt * 128 and M == nm * 128 and nm % 2 == 0
    return np.ascontiguousarray(
        w.reshape(nkt, 128, nm // 2, 2, 128).transpose(2, 1, 3, 0, 4))


def _tile_lhs(w, nm, nkt):
    """[K, M] -> [nm, 128, nkt, 128]."""
    K, M = w.shape
    assert K == nkt * 128 and M == nm * 128
    return np.ascontiguousarray(
        w.reshape(nkt, 128, nm, 128).transpose(2, 1, 0, 3))


def _tile_ff1_pairs(w):
    """[1024, 8192] -> [32, 128, 2, 8, 128]; [i][p][0]=gate tile 32+i, [i][p][1]=a tile i."""
    t = _tile_lhs(w, 64, 8)           # [64, 128, 8, 128]
    return np.ascontiguousarray(
        np.stack([t[32:64], t[0:32]], axis=2))


def _tile_rhs(w, nkt):
    K, N = w.shape
    assert K == nkt * 128 and N % 512 == 0
    return np.ascontiguousarray(
        w.reshape(nkt, 128, N // 512, 512).transpose(2, 1, 0, 3))


def _bias_cols(b, ncols):
    return np.ascontiguousarray(np.asarray(b, np.float32).reshape(ncols, 128).T)


_NC_CACHE = None


def kernel(**inputs):
    global _NC_CACHE
    inp = {k: np.asarray(v, np.float32) for k, v in inputs.items()}

    x = inp["x"][0]                    # [T, DIM]
    ctx = inp["context"][0]            # [77, CTX]
    xT_full = np.ascontiguousarray(x.T)
    ctxT = np.zeros((CTX, TCXP), np.float32)
    ctxT[:, :TCX] = ctx.T

    wq1 = np.ascontiguousarray((inp["n1_w"][:, None] * inp["q1_w"]) * SCALE)
    wk1 = np.ascontiguousarray(inp["n1_w"][:, None] * inp["k1_w"])
    wv1 = np.ascontiguousarray(inp["n1_w"][:, None] * inp["v1_w"])
    qb1 = (inp["n1_b"] @ inp["q1_w"]) * SCALE
    kb1 = inp["n1_b"] @ inp["k1_w"]
    vb1 = inp["n1_b"] @ inp["v1_w"]
    wq2 = np.ascontiguousarray((inp["n2_w"][:, None] * inp["q2_w"]) * SCALE)
    qb2 = (inp["n2_b"] @ inp["q2_w"]) * SCALE
    ff1 = np.ascontiguousarray(inp["n3_w"][:, None] * inp["ff1_w"])
    fb1 = inp["n3_b"] @ inp["ff1_w"] + inp["ff1_b"]

    sel16 = np.zeros((16, 8, 128), np.float32)
    for p in range(PAIRS):
        sel16[2 * p, p, 0:64] = 1.0
        sel16[2 * p + 1, p, 64:128] = 1.0

    shared = {
        "ctxT": ctxT,
        "sel16": sel16,
        "wq1t": _tile_lhs_pairs(wq1, 8, CKT),
        "wk1t": _tile_lhs_pairs(wk1, 8, CKT),
        "wv1t": _tile_rhs(wv1, CKT),
        "o1t": _tile_lhs_pairs(np.ascontiguousarray(inp["o1_w"]), 8, CKT),
        "wq2t": _tile_lhs_pairs(wq2, 8, CKT),
        "k2t": _tile_lhs_pairs(np.ascontiguousarray(inp["k2_w"]), 8, CKT_CTX),
        "v2t": _tile_rhs(np.ascontiguousarray(inp["v2_w"]), CKT_CTX),
        "o2t": _tile_lhs_pairs(np.ascontiguousarray(inp["o2_w"]), 8, CKT),
        "ff1t": _tile_ff1_pairs(ff1),
        "ff2t": _tile_lhs(np.ascontiguousarray(inp["ff2_w"]), 8, FF // 128),
        "vb1r": np.ascontiguousarray(vb1.reshape(1, DIM)),
    }
    f32_shared = {
        "qb1c": _bias_cols(qb1, 8),
        "kb1c": _bias_cols(kb1, 8),
        "o1bc": _bias_cols(inp["o1_b"], 8),
        "qb2c": _bias_cols(qb2, 8),
        "o2bc": _bias_cols(inp["o2_b"], 8),
        "fb1c": _bias_cols(fb1, 64),
        "ff2bc": _bias_cols(inp["ff2_b"], 8),
        "padmask": np.ascontiguousarray(
            (np.arange(128)[:, None] < TCX).astype(np.float32)
            * np.ones((1, 16), np.float32)),
    }
    shared = {k: np.ascontiguousarray(v, dtype=ml_dtypes.bfloat16)
              for k, v in shared.items()}
    shared.update({k: np.ascontiguousarray(v, dtype=np.float32)
                   for k, v in f32_shared.items()})

    in_maps = []
    for c in range(NCORES):
        m = dict(shared)
        m["xT"] = np.ascontiguousarray(xT_full[:, c * TO:(c + 1) * TO])
        m["rank_in"] = np.full((1, 1), float(c), np.float32)
        in_maps.append(m)

    if _NC_CACHE is None:
        _NC_CACHE = build_nc()
    nc = _NC_CACHE

    res = run_bass_kernel_spmd(nc, in_maps, core_ids=list(range(NCORES)))

    outs = [res.results[c]["outT"].T for c in range(NCORES)]   # each [TO, DIM]
    return np.ascontiguousarray(np.concatenate(outs, axis=0))[None].astype(np.float32)


if __name__ == "__main__":
    d = np.load("/tmp/ref_inputs.npz")
    out = kernel(**{k: d[k] for k in d.files})
    ref = np.load("/tmp/ref_out.npy")
    err = np.abs(out - ref).max()
    print("max abs err:", err, " absmax ref:", np.abs(ref).max(),
          " rel:", err / np.abs(ref).max())


# revision 30
# speedup vs baseline: 1.1442x; 1.0509x over previous
"""Trainium2 Bass kernel for nn_BasicTransformerBlock (self-attn + cross-attn
+ GEGLU FF, dim=1024, heads=16, seq=4096, ctx=77).

Strategy (8 NeuronCores), v2:
 - Sequence-parallel: each core owns 512 tokens end-to-end, activations kept
   transposed on-chip ([channel, token]).
 - K and augmented-V for self-attention are fp8 (e4m3); one fused AllGather
   ships both (half the wire bytes of the bf16 baseline). The AG bubble is
   filled with the Q projection, the cross-attn K2/V2 projections, and the
   "diagonal" score+exp prefill (own-rank key tiles from SBUF).
 - Softmax without max-subtraction. exp is split across ScalarE (table exp,
   fp8 out) and VectorE (integer Schraudolph: bits = round(s*8/ln2 + 8*(7-C))
   written as uint8, bit-identical to fp8e4m3) - the two engines each handle
   ~half of the 33M exps, which was the baseline's critical path.
 - P^T and V both fp8 => AV matmuls run in DoubleRow mode (256-deep virtual
   contraction), halving the attention AV matmul count. The V rows carry a
   trailing ones column per head so softmax denominators fall out of the AV
   matmul for free.
 - All row broadcasts are matmuls on TensorE: LN scale/shift via K=1
   ones-row matmuls, softmax 1/z via one batched 16-lane reciprocal + K=16
   selector matmuls (a [1,512] DVE reciprocal costs 3.3us; [16,512] costs
   the same).
 - Weights stay bf16 (fp8 weights push rel-err to ~2e-2). Weight DMAs are
   batched in 2-m-tile pairs on the sync HWDGE queue; FF1/FF2 prefetch on
   the scalar queue; AG-gated gathered-K/V loads and z-row gathers ride the
   GpSimd SWDGE queue so no AG-gated DMA can head-block weight streaming
   (the Tile scheduler reorders queues, which otherwise froze the sync
   queue behind the collective).
 - Attention inner loop is software-pipelined one rank deep (scores+exp of
   rank r overlap the AV matmuls of rank r-1) so the in-order PE queue
   never stalls waiting for exp results.
Host gathers the 8 transposed output shards and transposes back.
"""
import numpy as np
import ml_dtypes
from contextlib import ExitStack

import concourse.bass as bass
import concourse.tile as tile
import concourse.mybir as mybir
from concourse.bass_utils import run_bass_kernel_spmd


# --- inlined BIR sync-wait legalizer (toolchain accepts max 1 wait/inst) ---
import json as _json


def _legalize_bir_json(raw, max_waits=1):
    d = _json.loads(raw)
    ctr = 0
    for f in d.get("functions", []):
        for bb in f.get("blocks", []):
            out = []
            for ins in bb.get("instructions", []):
                si = ins.get("sync_info")
                if si:
                    waits = si.get("on_wait") or []
                    if len(waits) > max_waits:
                        extra, keep = waits[:-max_waits], waits[-max_waits:]
                        for w in extra:
                            ctr += 1
                            out.append({
                                "debug": ins.get("debug", 0),
                                "engine": ins["engine"],
                                "ins": [],
                                "outs": [],
                                "name": f"waitfix-{ctr}",
                                "opcode": "EventSemaphore",
                                "sync_info": {"on_update": [], "on_wait": [w]},
                            })
                        si["on_wait"] = keep
                    ups = si.get("on_update") or []
                    if len(ups) > 1:
                        raise AssertionError(
                            f"instruction {ins.get('name')} has {len(ups)} updates")
                out.append(ins)
            bb["instructions"] = out
    return _json.dumps(d).encode()


def _install_legalizer(max_waits=1):
    import concourse.bass as _bassmod

    if getattr(_bassmod.Bass, "_legalize_installed", False):
        return
    orig = _bassmod.Bass.to_json_bytes

    def patched(self):
        return _legalize_bir_json(orig(self), max_waits=max_waits)

    _bassmod.Bass.to_json_bytes = patched
    _bassmod.Bass._legalize_installed = True


_install_legalizer()


def _install_ldwopt():
    import concourse.bass_utils as _bu

    if getattr(_bu, "_ldwopt_patched", False):
        return
    _orig_rc = _bu.run_command

    def _rc(cmd, **kw):
        try:
            cmd = ["--enable-ldw-opt=true" if c == "--enable-ldw-opt=false"
                   else c for c in cmd]
        except TypeError:
            pass
        return _orig_rc(cmd, **kw)

    _bu.run_command = _rc
    _bu._ldwopt_patched = True


# NOTE: --enable-ldw-opt=true rejects DoubleRow InstLdweights; left disabled.
# _install_ldwopt()

F32 = mybir.dt.float32
F32R = mybir.dt.float32r
BF16 = mybir.dt.bfloat16
F8 = mybir.dt.float8e4
U8 = mybir.dt.uint8
AF = mybir.ActivationFunctionType
OP = mybir.AluOpType
DR = mybir.MatmulPerfMode.DoubleRow

DIM = 1024
HEADS = 16
D = 64
CTX = 768
FF = 4096
T = 4096
NCORES = 8
TO = T // NCORES          # 512 own tokens per core
PAIRS = HEADS // 2        # 8 head pairs
CKT = DIM // 128          # 8 contraction tiles over DIM
CKT_CTX = CTX // 128      # 6 contraction tiles over CTX
TCX = 77
TCXP = 80                 # ctx tokens padded
SCALE = D ** -0.5
EPS = 1e-5

# V augmented row layout (fp8): per pair a 160-col block:
#   [0:64)  V head A   [64] ones A   [65:80) pad
#   [80:144) V head B  [144] ones B  [145:160) pad
PB = 160                     # pair block width
V_ROWP = PAIRS * PB          # 1280
K_ELEMS = DIM * TO           # 524288 fp8 bytes
V_ELEMS = TO * V_ROWP        # 655360
AG_ELEMS = K_ELEMS + V_ELEMS

# Schraudolph fp8 exp constants (validated bit-exact vs HW probe)
LOG2E = 1.4426950408889634
SCH_C = 0.0430
SCH_A8 = 8.0 * LOG2E
SCH_B8 = 8.0 * (7.0 - SCH_C)


def _ap(tensor_ap, offset, steps):
    """Raw AP view on a (flat) dram tensor: steps = [[step, count], ...]."""
    return bass.AP(tensor=tensor_ap.tensor, offset=tensor_ap.offset + offset,
                   ap=list(steps))


def build_nc(fake_ag=False):
    nc = bass.Bass(trn_type="TRN2")

    # ---- dram tensors ----------------------------------------------------
    xT = nc.dram_tensor("xT", [DIM, TO], F32, kind="ExternalInput")
    ctxT = nc.dram_tensor("ctxT", [CTX, TCXP], BF16, kind="ExternalInput")

    def w_in(name, shape=None, dt=BF16, shape_=None):
        return nc.dram_tensor(name, list(shape if shape is not None else shape_),
                              dt, kind="ExternalInput")

    # paired m-tile layouts: [nm/2, 128, 2, nkt, 128]
    wq1t = w_in("wq1t", (4, 128, 2, CKT, 128))
    wk1t = w_in("wk1t", (4, 128, 2, CKT, 128))
    wv1t = w_in("wv1t", (2, 128, CKT, 512))
    o1t = w_in("o1t", (4, 128, 2, CKT, 128))
    wq2t = w_in("wq2t", (4, 128, 2, CKT, 128))
    k2t = w_in("k2t", (4, 128, 2, CKT_CTX, 128))
    v2t = w_in("v2t", (2, 128, CKT_CTX, 512))
    o2t = w_in("o2t", (4, 128, 2, CKT, 128))
    ff1t = w_in("ff1t", (32, 128, 2, CKT, 128))   # [i] = (gate 32+i, a i)
    ff2t = w_in("ff2t", (8, 128, FF // 128, 128))

    sel16d = w_in("sel16", dt=BF16, shape_=(16, 8, 128))
    qb1c = w_in("qb1c", dt=F32, shape_=(128, 8))
    kb1c = w_in("kb1c", dt=F32, shape_=(128, 8))
    vb1r = w_in("vb1r", dt=BF16, shape_=(1, DIM))
    o1bc = w_in("o1bc", dt=F32, shape_=(128, 8))
    qb2c = w_in("qb2c", dt=F32, shape_=(128, 8))
    o2bc = w_in("o2bc", dt=F32, shape_=(128, 8))
    fb1c = w_in("fb1c", dt=F32, shape_=(128, 64))
    padmask = w_in("padmask", dt=F32, shape_=(128, 16))
    ff2bc = w_in("ff2bc", dt=F32, shape_=(128, 8))

    outT = nc.dram_tensor("outT", [DIM, TO], F32, kind="ExternalOutput")

    with tile.TileContext(nc) as tc, ExitStack() as top:
        dram = top.enter_context(tc.tile_pool(name="dram", bufs=1, space="DRAM"))
        p_const = top.enter_context(tc.tile_pool(name="p_const", bufs=1))

        # ---- constants ---------------------------------------------------
        ones_col_f = p_const.tile([128, 1], F32, name="ones_col_f")
        nc.vector.memset(ones_col_f[:], 1.0)
        ones_col = p_const.tile([128, 1], F32R, name="ones_col")
        nc.scalar.copy(ones_col[:], ones_col_f[:])
        ones_row_bf = p_const.tile([1, 128], BF16, name="ones_row_bf")
        nc.vector.memset(ones_row_bf[:], 1.0)
        # sel16[:, p, :]: K=16 selector that broadcasts 1/z rows 2p (to
        # partitions 0:64) and 2p+1 (to 64:128)
        sel16 = p_const.tile([16, 8, 128], BF16, name="sel16")
        nc.sync.dma_start(out=sel16, in_=sel16d.ap())
        padones = p_const.tile([128, 16], F32, name="padones")
        nc.sync.dma_start(out=padones, in_=padmask.ap())
        eps_row = p_const.tile([1, 1], F32, name="eps_row")
        nc.vector.memset(eps_row[:], EPS)

        def bias_tile(name, dram_t, cols):
            t = p_const.tile([128, cols], F32, name=name)
            nc.sync.dma_start(out=t, in_=dram_t.ap())
            return t

        qb1 = bias_tile("qb1", qb1c, 8)
        kb1 = bias_tile("kb1", kb1c, 8)
        o1b = bias_tile("o1b", o1bc, 8)
        qb2 = bias_tile("qb2", qb2c, 8)
        o2b = bias_tile("o2b", o2bc, 8)
        fb1 = bias_tile("fb1", fb1c, 64)
        ff2b = bias_tile("ff2b", ff2bc, 8)
        # vb1 broadcast via K=1 matmul at V-evac time needs [128, DIM] view;
        # build it once into SBUF from a psum broadcast.
        vb1row = p_const.tile([1, DIM], BF16, name="vb1row")
        nc.sync.dma_start(out=vb1row, in_=vb1r.ap())
        vb1bc = p_const.tile([128, DIM], F32, name="vb1bc")
        with ExitStack() as st0:
            psb0 = st0.enter_context(tc.tile_pool(name="psb0", bufs=1, space="PSUM"))
            pv = psb0.tile([128, 512], F32, name="pv", tag="pv")
            for half in range(2):
                nc.tensor.matmul(pv[:], ones_row_bf[:],
                                 vb1row[:, half * 512:(half + 1) * 512],
                                 start=True, stop=True)
                nc.vector.tensor_copy(vb1bc[:, half * 512:(half + 1) * 512], pv[:])
        ctx_sb = []
        for i in range(CKT_CTX):
            t = p_const.tile([128, TCXP], BF16, name=f"ctxsb{i}")
            nc.sync.dma_start(out=t, in_=ctxT.ap()[i * 128:(i + 1) * 128, :])
            ctx_sb.append(t)

        # exp engine alternation counter
        exp_state = {"i": 0}

        def exp_to_fp8(out_ap, in_ap):
            """exp of a PSUM tile into an fp8 SBUF AP; alternates ACT/DVE."""
            if exp_state["i"] % 2 == 0:
                nc.scalar.activation(out_ap, in_ap, AF.Exp)
            else:
                nc.vector.tensor_scalar(out_ap.bitcast(U8), in_ap,
                                        SCH_A8, SCH_B8, op0=OP.mult, op1=OP.add)
            exp_state["i"] += 1

        def exp_split(out4, lt, pss):
            """exp of one [128,2,TO] score tile: head A on ScalarE (table),
            head B on VectorE (integer Schraudolph). Half the latency of a
            single-engine pass, so score PSUM frees faster."""
            nc.scalar.activation(out4[:, lt, 0, :], pss[:, 0, :], AF.Exp)
            nc.vector.tensor_scalar(out4[:, lt, 1, :].bitcast(U8), pss[:, 1, :],
                                    SCH_A8, SCH_B8, op0=OP.mult, op1=OP.add)

        def exp_to_bf16(out_ap, in_ap):
            if exp_state["i"] % 2 == 0:
                nc.scalar.activation(out_ap, in_ap, AF.Exp)
            else:
                nc.vector.tensor_scalar(out_ap.bitcast(mybir.dt.uint16), in_ap,
                                        128.0 * LOG2E, 128.0 * (127.0 - SCH_C),
                                        op0=OP.mult, op1=OP.add)
            exp_state["i"] += 1

        # ---- helpers -----------------------------------------------------
        def layernorm(xtiles, h_pool, tag, out_dtype=BF16):
            """xtiles: 8 sbuf tiles [128, TO] F32R -> 8 out tiles [128,TO]."""
            with ExitStack() as ln:
                work = ln.enter_context(tc.tile_pool(name=f"lnw_{tag}", bufs=2))
                rows = ln.enter_context(tc.tile_pool(name=f"lnr_{tag}", bufs=1))
                ps = ln.enter_context(tc.tile_pool(name=f"lnp_{tag}", bufs=1,
                                                   space="PSUM"))
                ps_s = ps.tile([1, TO], F32, name=f"pss_{tag}", tag="s")
                ps_q = ps.tile([1, TO], F32, name=f"psq_{tag}", tag="q")
                for i in range(8):
                    sq = work.tile([128, TO], F32R, name=f"sq_{tag}", tag="sq")
                    nc.gpsimd.tensor_tensor(sq[:], xtiles[i].bitcast(F32),
                                            xtiles[i].bitcast(F32), op=OP.mult)
                    nc.tensor.matmul(ps_s[:], ones_col[:], xtiles[i][:],
                                     start=(i == 0), stop=(i == 7))
                    nc.tensor.matmul(ps_q[:], ones_col[:], sq[:],
                                     start=(i == 0), stop=(i == 7))
                mu = rows.tile([1, TO], F32, name=f"mu_{tag}")
                nc.vector.tensor_scalar(mu[:], ps_s[:], 1.0 / DIM, None, op0=OP.mult)
                m2 = rows.tile([1, TO], F32, name=f"m2_{tag}")
                nc.vector.tensor_scalar(m2[:], ps_q[:], 1.0 / DIM, None, op0=OP.mult)
                var = rows.tile([1, TO], F32, name=f"var_{tag}")
                nc.vector.tensor_tensor(var[:], mu[:], mu[:], op=OP.mult)
                nc.vector.tensor_tensor(var[:], m2[:], var[:], op=OP.subtract)
                # 1/sqrt(v+eps) = exp(-0.5*ln(v+eps)); table ops beat the
                # 3.3us single-row DVE reciprocal
                lnv = rows.tile([1, TO], F32, name=f"lnv_{tag}")
                nc.scalar.activation(lnv[:], var[:], AF.Ln, bias=eps_row[:])
                ra = rows.tile([1, TO], F32, name=f"ra_{tag}")
                nc.scalar.activation(ra[:], lnv[:], AF.Exp, scale=-0.5)
                rb = rows.tile([1, TO], F32, name=f"rb_{tag}")
                nc.vector.scalar_tensor_tensor(rb[:], mu[:], -1.0, ra[:],
                                               op0=OP.mult, op1=OP.mult)
                rab = rows.tile([1, TO], BF16, name=f"rab_{tag}")
                nc.scalar.copy(rab[:], ra[:])
                rbb = rows.tile([1, TO], BF16, name=f"rbb_{tag}")
                nc.scalar.copy(rbb[:], rb[:])
                ps_a = ps.tile([128, TO], F32, name=f"psa_{tag}", tag="a")
                ps_b = ps.tile([128, TO], F32, name=f"psb_{tag}", tag="b")
                nc.tensor.matmul(ps_a[:], ones_row_bf[:], rab[:],
                                 start=True, stop=True)
                nc.tensor.matmul(ps_b[:], ones_row_bf[:], rbb[:],
                                 start=True, stop=True)
                bsb = rows.tile([128, TO], F32, name=f"bsb_{tag}")
                nc.vector.tensor_copy(bsb[:], ps_b[:])
                out = []
                for i in range(8):
                    tmp = work.tile([128, TO], F32, name=f"tmp_{tag}", tag="tmp")
                    nc.vector.tensor_tensor(tmp[:], xtiles[i].bitcast(F32),
                                            ps_a[:], op=OP.mult)
                    h = h_pool.tile([128, TO], out_dtype, name=f"h_{tag}{i}")
                    nc.gpsimd.tensor_tensor(h[:], tmp[:], bsb[:], op=OP.add)
                    out.append(h)
                return out

        def proj_T(wdram, rhs_tiles, bias, out_pool, tag, nkt=CKT,
                   out_dtype=BF16, residual=None, res_bias=None, nmb=4,
                   store_dram=None):
            """Paired m-tile projection. wdram: [nmb, 128, 2, nkt, 128]."""
            outs = []
            with ExitStack() as st:
                wp = st.enter_context(tc.tile_pool(name=f"wp_{tag}", bufs=2))
                ps = st.enter_context(tc.tile_pool(name=f"ps_{tag}", bufs=2,
                                                   space="PSUM"))
                wtiles = []
                for mb in range(min(2, nmb)):
                    wm = wp.tile([128, 2, nkt, 128], BF16, name=f"wm_{tag}", tag="w")
                    nc.sync.dma_start(out=wm, in_=wdram.ap()[mb])
                    wtiles.append(wm)
                for mb in range(nmb):
                    wm = wtiles[mb]
                    for j in range(2):
                        m = 2 * mb + j
                        psy = ps.tile([128, TO], F32, name=f"psy_{tag}", tag="y")
                        for kt in range(nkt):
                            nc.tensor.matmul(psy[:], wm[:, j, kt, :],
                                             rhs_tiles[kt][:],
                                             start=(kt == 0), stop=(kt == nkt - 1))
                        o = out_pool.tile([128, TO], out_dtype, name=f"o_{tag}{m}")
                        if residual is not None:
                            nc.vector.scalar_tensor_tensor(
                                o[:], psy[:], res_bias[:, m:m + 1],
                                residual[m].bitcast(F32), op0=OP.add, op1=OP.add)
                        elif bias is not None:
                            nc.vector.tensor_scalar(o[:], psy[:], bias[:, m:m + 1],
                                                    None, op0=OP.add)
                        else:
                            nc.vector.tensor_copy(o[:], psy[:])
                        if store_dram is not None:
                            nc.sync.dma_start(
                                out=_ap(store_dram, m * 128 * TO,
                                        [[TO, 128], [1, TO]]),
                                in_=o[:])
                        outs.append(o)
                    if mb + 2 < nmb:
                        wn = wp.tile([128, 2, nkt, 128], BF16, name=f"wm_{tag}",
                                     tag="w")
                        nc.sync.dma_start(out=wn, in_=wdram.ap()[mb + 2])
                        wtiles.append(wn)
            return outs

        # ---- AG buffer (fused K + augmented V, fp8) ----------------------
        agkv_in = dram.tile([AG_ELEMS], F8, name="agkv_in")
        agkv_out = dram.tile([NCORES * AG_ELEMS], F8, name="agkv_out",
                             addr_space="Local" if fake_ag else "Shared")

        # ================= phase A: LN1 + K/V/Q projections ===============
        p_xT = top.enter_context(tc.tile_pool(name="p_xT", bufs=1))
        p_QT = top.enter_context(tc.tile_pool(name="p_QT", bufs=1))
        p_OT = top.enter_context(tc.tile_pool(name="p_OT", bufs=1))
        p_kv = top.enter_context(tc.tile_pool(name="p_kv", bufs=1))

        xtiles = []
        for i in range(8):
            t = p_xT.tile([128, TO], F32R, name=f"xT{i}")
            nc.sync.dma_start(out=t,
                              in_=xT.ap()[i * 128:(i + 1) * 128, :].bitcast(F32R))
            xtiles.append(t)

        with ExitStack() as phA:
            p_h1 = phA.enter_context(tc.tile_pool(name="p_h1", bufs=1))
            h1 = layernorm(xtiles, p_h1, "ln1")

            # K^T own (fp8) -> SBUF (for diagonal) + agkv_in rows [0 : DIM)
            ko = proj_T(wk1t, h1, kb1, p_kv, "k1", out_dtype=F8,
                        store_dram=agkv_in[:])

            # V own augmented (fp8) -> vag4 [128, 4, V_ROWP] + agkv_in
            vag4 = p_kv.tile([128, 4, PAIRS, PB], F8, name="vag4")
            with ExitStack() as stv:
                wvp = stv.enter_context(tc.tile_pool(name="wp_v1", bufs=1))
                ps = stv.enter_context(tc.tile_pool(name="ps_v1", bufs=2,
                                                    space="PSUM"))
                wv_sb = []
                for nb in range(2):
                    w = wvp.tile([128, CKT, 512], BF16, name=f"wv{nb}")
                    nc.sync.dma_start(out=w, in_=wv1t.ap()[nb])
                    wv_sb.append(w)
                for t4 in range(4):
                    for nb in range(2):
                        psv = ps.tile([128, 512], F32, name="psv", tag="v")
                        for kt in range(CKT):
                            nc.tensor.matmul(
                                psv[:], h1[kt][:, t4 * 128:(t4 + 1) * 128],
                                wv_sb[nb][:, kt, :],
                                start=(kt == 0), stop=(kt == CKT - 1))
                        # psv inner layout: 8 heads x 64; heads hh=0..7 map to
                        # pair pp = 4*nb + hh//2, half b = hh%2 at col 80b..80b+64
                        psv3 = psv.rearrange("p (g two e) -> p g two e", two=2, e=D)
                        vslice = vag4[:, t4, 4 * nb:4 * nb + 4, :]
                        for b in range(2):
                            nc.vector.tensor_tensor(
                                vslice[:, :, 80 * b:80 * b + D],
                                psv3[:, :, b, :],
                                vb1bc[:, nb * 512:(nb + 1) * 512].rearrange(
                                    "p (g two e) -> p g two e", two=2, e=D)[:, :, b, :],
                                op=OP.add)
                # ones columns (denominator rows) for both halves of all pairs
                for b in range(2):
                    nc.vector.memset(vag4[:, :, :, 80 * b + D:80 * b + D + 1], 1.0)
                nc.sync.dma_start(
                    out=_ap(agkv_in[:], K_ELEMS,
                            [[V_ROWP, 128], [128 * V_ROWP, 4], [1, V_ROWP]]),
                    in_=vag4[:])

            if fake_ag:
                for r in range(NCORES):
                    nc.sync.dma_start(
                        out=_ap(agkv_out[:], r * AG_ELEMS, [[1, AG_ELEMS]]),
                        in_=_ap(agkv_in[:], 0, [[1, AG_ELEMS]]))
            else:
                nc.gpsimd.collective_compute(
                    "AllGather", OP.bypass,
                    replica_groups=[list(range(NCORES))],
                    ins=[agkv_in[:]], outs=[agkv_out[:]])

            QT = proj_T(wq1t, h1, qb1, p_QT, "q1", out_dtype=F8)

        # ---- cross-attn K2/V2 from context (fills the AG bubble) ---------
        p_kv2 = top.enter_context(tc.tile_pool(name="p_kv2", bufs=1))
        K2T = []
        with ExitStack() as stk2:
            wp = stk2.enter_context(tc.tile_pool(name="wp_k2", bufs=2))
            ps = stk2.enter_context(tc.tile_pool(name="ps_k2", bufs=2, space="PSUM"))
            for mb in range(4):
                wm = wp.tile([128, 2, CKT_CTX, 128], BF16, name="wm_k2", tag="w")
                nc.sync.dma_start(out=wm, in_=k2t.ap()[mb])
                for j in range(2):
                    psy = ps.tile([128, TCXP], F32, name="psy_k2", tag="y")
                    for kt in range(CKT_CTX):
                        nc.tensor.matmul(psy[:], wm[:, j, kt, :], ctx_sb[kt][:],
                                         start=(kt == 0), stop=(kt == CKT_CTX - 1))
                    k2 = p_kv2.tile([128, TCXP], BF16, name=f"k2_{2 * mb + j}")
                    nc.vector.tensor_copy(k2[:], psy[:])
                    K2T.append(k2)

        V2_ROW = HEADS * (D + 1)  # 1040, ones at end of each 65-block
        v2ag = p_kv2.tile([TCXP, V2_ROW], BF16, name="v2ag")
        v2ag3 = v2ag.rearrange("p (h e) -> p h e", e=D + 1)
        with ExitStack() as stv2:
            wvp = stv2.enter_context(tc.tile_pool(name="wp_v2", bufs=1))
            ps = stv2.enter_context(tc.tile_pool(name="ps_v2", bufs=2, space="PSUM"))
            for nb in range(2):
                w = wvp.tile([128, CKT_CTX, 512], BF16, name=f"wv2_{nb}", tag="w")
                nc.sync.dma_start(out=w, in_=v2t.ap()[nb])
                psv = ps.tile([TCXP, 512], F32, name="psv2", tag="v")
                for kt in range(CKT_CTX):
                    nc.tensor.matmul(psv[:], ctx_sb[kt][:], w[:, kt, :],
                                     start=(kt == 0), stop=(kt == CKT_CTX - 1))
                nc.vector.tensor_copy(
                    v2ag3[:, nb * 8:(nb + 1) * 8, 0:D],
                    psv[:].rearrange("p (h e) -> p h e", e=D))
            nc.scalar.copy(v2ag3[:, :, D:D + 1], padones[0:TCXP, :].unsqueeze(2))

        # ================= phase B: self-attention ========================
        p_ff1w = top.enter_context(tc.tile_pool(name="p_ff1w", bufs=8))
        p_ff2w = top.enter_context(tc.tile_pool(name="p_ff2w", bufs=2))

        with ExitStack() as phB:
            ps_S = phB.enter_context(tc.tile_pool(name="ps_S", bufs=3, space="PSUM"))
            ps_AV = phB.enter_context(tc.tile_pool(name="ps_AV", bufs=1,
                                                   space="PSUM"))
            p_at = phB.enter_context(tc.tile_pool(name="p_at", bufs=2))
            p_pt = phB.enter_context(tc.tile_pool(name="p_pt", bufs=3))
            p_rb = phB.enter_context(tc.tile_pool(name="p_rb", bufs=2))

            def score_step(pss, lhsA, lhsB, qt):
                nc.tensor.matmul(pss[:, 0, :], lhsA, qt[0:64, :],
                                 start=True, stop=True, tile_position=(0, 0))
                nc.tensor.matmul(pss[:, 1, :], lhsB, qt[64:128, :],
                                 start=True, stop=True, tile_position=(64, 0))

            # --- bubble fill: own-token scores+exp warm-up from SBUF ------
            # (SPMD program cannot branch on rank, so these exps are
            # recomputed from the gathered buffer below; this pass just keeps
            # PE/ACT/DVE warm while the AllGather is in flight.)
            for p in range(PAIRS):
                pt = p_pt.tile([128, 4, 2, TO], F8, name="ptm", tag="pt")
                for lt in range(4):
                    pss = ps_S.tile([128, 2, TO], F32, name="pss", tag="s")
                    score_step(pss,
                               ko[p][0:64, lt * 128:(lt + 1) * 128],
                               ko[p][64:128, lt * 128:(lt + 1) * 128],
                               QT[p])
                    exp_to_fp8(pt[:, lt, :, :], pss[:])

            # --- prefetch first FF1 weight pairs on the scalar queue ------
            ff1_tiles = [None] * 32
            for i in range(8):
                wt = p_ff1w.tile([128, 2, CKT, 128], BF16, name="wff1", tag="w")
                nc.scalar.dma_start(out=wt, in_=ff1t.ap()[i])
                ff1_tiles[i] = wt

            # --- main loop ------------------------------------------------
            zsb = p_rb.tile([16, TO], F32, name="zsb", bufs=1)
            OT = []
            def load_pair(p):
                kpair = p_at.tile([128, NCORES, TO], F8, name="kpair", tag="kp")
                nc.gpsimd.dma_start(
                    out=kpair[:],
                    in_=_ap(agkv_out[:], p * 128 * TO,
                            [[TO, 128], [AG_ELEMS, NCORES], [1, TO]]))
                vpr = p_at.tile([128, NCORES, 4, PB], F8, name="vpr", tag="vp")
                for r in range(NCORES):
                    nc.gpsimd.dma_start(
                        out=vpr[:, r, :, :],
                        in_=_ap(agkv_out[:], r * AG_ELEMS + K_ELEMS + p * PB,
                                [[V_ROWP, 128], [128 * V_ROWP, 4], [1, PB]]))
                return kpair, vpr

            loaded = {0: load_pair(0)}
            for p in range(PAIRS):
                kpair, vpr = loaded.pop(p)
                if p + 1 < PAIRS:
                    loaded[p + 1] = load_pair(p + 1)

                psA = ps_AV.tile([65, 2, TO], F32, name="psA", tag="A")

                def av_step(r, ptm):
                    for i in range(2):
                        nc.tensor.matmul(
                            psA[:, 0, :],
                            vpr[:, r, 2 * i:2 * i + 2, 0:D + 1],
                            ptm[:, 2 * i:2 * i + 2, 0, :],
                            start=(r == 0 and i == 0),
                            stop=(r == NCORES - 1 and i == 1),
                            perf_mode=DR)
                        nc.tensor.matmul(
                            psA[:, 1, :],
                            vpr[:, r, 2 * i:2 * i + 2, 80:80 + D + 1],
                            ptm[:, 2 * i:2 * i + 2, 1, :],
                            start=(r == 0 and i == 0),
                            stop=(r == NCORES - 1 and i == 1),
                            perf_mode=DR)

                # software pipeline: scores+exp for rank r while AV of r-1
                # runs, so the in-order PE queue never stalls on exp results
                ptm_prev = None
                for r in range(NCORES):
                    ptm = p_pt.tile([128, 4, 2, TO], F8, name="ptm", tag="pt")
                    for lt in range(4):
                        pss = ps_S.tile([128, 2, TO], F32, name="pss", tag="s")
                        score_step(pss,
                                   kpair[0:64, r, lt * 128:(lt + 1) * 128],
                                   kpair[64:128, r, lt * 128:(lt + 1) * 128],
                                   QT[p])
                        exp_to_fp8(ptm[:, lt, :, :], pss[:])
                    if ptm_prev is not None:
                        av_step(r - 1, ptm_prev)
                    ptm_prev = ptm
                av_step(NCORES - 1, ptm_prev)

                # stash unnormalized AV + z rows; batched normalize below
                zrA = p_rb.tile([1, TO], F32, name="zrA", tag="zra")
                nc.vector.tensor_copy(zrA[:], psA[D:D + 1, 0, :])
                nc.gpsimd.dma_start(out=zsb[2 * p:2 * p + 1, :], in_=zrA[:])
                zrB = p_rb.tile([1, TO], F32, name="zrB", tag="zrb")
                nc.scalar.copy(zrB[:], psA[D:D + 1, 1, :])
                nc.gpsimd.dma_start(out=zsb[2 * p + 1:2 * p + 2, :], in_=zrB[:])
                ot = p_OT.tile([128, TO], BF16, name=f"ot{p}")
                nc.vector.tensor_copy(ot[0:64, :], psA[0:D, 0, :])
                nc.scalar.copy(ot[64:128, :], psA[0:D, 1, :])
                OT.append(ot)

            # batched softmax divide: one 16-lane reciprocal, then per-pair
            # K=16 selector matmul broadcast + in-place multiply
            zrec = p_rb.tile([16, TO], F32, name="zrec", bufs=1)
            nc.vector.reciprocal(zrec[:], zsb[:])
            zrecb = p_rb.tile([16, TO], BF16, name="zrecb", bufs=1)
            nc.scalar.copy(zrecb[:], zrec[:])
            for p in range(PAIRS):
                psbc2t = ps_S.tile([128, 2, TO], F32, name="pss", tag="s")
                psbc = psbc2t[:, 0, :]
                nc.tensor.matmul(psbc, sel16[:, p, :], zrecb[:],
                                 start=True, stop=True)
                bcz = p_rb.tile([128, TO], BF16, name="bcz", tag="bcz")
                nc.vector.tensor_copy(bcz[:], psbc)
                nc.vector.tensor_tensor(OT[p][:], OT[p][:], bcz[:], op=OP.mult)

        # o1 projection + residual -> x2T
        p_x2 = top.enter_context(tc.tile_pool(name="p_x2", bufs=1))
        x2T = proj_T(o1t, OT, None, p_x2, "o1", residual=xtiles, res_bias=o1b,
                     out_dtype=F32R)

        # ================= phase C: cross-attention =======================
        p_x3 = top.enter_context(tc.tile_pool(name="p_x3", bufs=1))
        with ExitStack() as phC:
            p_Q2 = phC.enter_context(tc.tile_pool(name="p_Q2", bufs=1))
            p_OT2 = phC.enter_context(tc.tile_pool(name="p_OT2", bufs=1))

            with ExitStack() as stc:
                p_h2 = stc.enter_context(tc.tile_pool(name="p_h2", bufs=1))
                h2 = layernorm(x2T, p_h2, "ln2")
                Q2T = proj_T(wq2t, h2, qb2, p_Q2, "q2")

            with ExitStack() as stx:
                ps_S2 = stx.enter_context(tc.tile_pool(name="ps_S2", bufs=2,
                                                       space="PSUM"))
                ps_A2 = stx.enter_context(tc.tile_pool(name="ps_A2", bufs=1,
                                                       space="PSUM"))
                ps_B2 = stx.enter_context(tc.tile_pool(name="ps_B2", bufs=2,
                                                       space="PSUM"))
                p_pt2 = stx.enter_context(tc.tile_pool(name="p_pt2", bufs=2))
                p_rb2 = stx.enter_context(tc.tile_pool(name="p_rb2", bufs=2))
                zsb2 = p_rb2.tile([16, TO], F32, name="zsb2", bufs=1)
                OT2 = []
                for p in range(PAIRS):
                    pss = ps_S2.tile([TCXP, 2, TO], F32, name="pss2", tag="s")
                    nc.tensor.matmul(pss[:, 0, :], K2T[p][0:64, :], Q2T[p][0:64, :],
                                     start=True, stop=True, tile_position=(0, 0))
                    nc.tensor.matmul(pss[:, 1, :], K2T[p][64:128, :],
                                     Q2T[p][64:128, :],
                                     start=True, stop=True, tile_position=(64, 0))
                    pt = p_pt2.tile([TCXP, 2, TO], BF16, name="pt2", tag="pt")
                    exp_to_bf16(pt[:], pss[:])
                    psA = ps_A2.tile([D + 1, 2, TO], F32, name="psA2c", tag="A")
                    nc.tensor.matmul(psA[:, 0, :],
                                     v2ag[:, (2 * p) * (D + 1):(2 * p + 1) * (D + 1)],
                                     pt[:, 0, :], start=True, stop=True)
                    nc.tensor.matmul(psA[:, 1, :],
                                     v2ag[:, (2 * p + 1) * (D + 1):(2 * p + 2) * (D + 1)],
                                     pt[:, 1, :], start=True, stop=True)
                    zrA = p_rb2.tile([1, TO], F32, name="zrAc", tag="zra")
                    nc.vector.tensor_copy(zrA[:], psA[D:D + 1, 0, :])
                    nc.gpsimd.dma_start(out=zsb2[2 * p:2 * p + 1, :], in_=zrA[:])
                    zrB = p_rb2.tile([1, TO], F32, name="zrBc", tag="zrb")
                    nc.vector.tensor_copy(zrB[:], psA[D:D + 1, 1, :])
                    nc.gpsimd.dma_start(out=zsb2[2 * p + 1:2 * p + 2, :], in_=zrB[:])
                    ot = p_OT2.tile([128, TO], BF16, name=f"ot2_{p}")
                    nc.vector.tensor_copy(ot[0:64, :], psA[0:D, 0, :])
                    nc.scalar.copy(ot[64:128, :], psA[0:D, 1, :])
                    OT2.append(ot)

                zrec2 = p_rb2.tile([16, TO], F32, name="zrec2", bufs=1)
                nc.vector.reciprocal(zrec2[:], zsb2[:])
                zrecb2 = p_rb2.tile([16, TO], BF16, name="zrecb2", bufs=1)
                nc.scalar.copy(zrecb2[:], zrec2[:])
                for p in range(PAIRS):
                    psbc = ps_B2.tile([128, TO], F32, name="psbc2", tag="bc")
                    nc.tensor.matmul(psbc[:], sel16[:, p, :], zrecb2[:],
                                     start=True, stop=True)
                    bcz = p_rb2.tile([128, TO], BF16, name="bczc", tag="bcz")
                    nc.scalar.copy(bcz[:], psbc[:])
                    nc.vector.tensor_tensor(OT2[p][:], OT2[p][:], bcz[:],
                                            op=OP.mult)

            x3T = proj_T(o2t, OT2, None, p_x3, "o2", residual=x2T, res_bias=o2b,
                         out_dtype=F32R)

        # ================= phase D: GEGLU FF ==============================
        with ExitStack() as phD:
            p_hT = phD.enter_context(tc.tile_pool(name="p_hT", bufs=1))
            hT = []
            with ExitStack() as stf:
                p_h3 = stf.enter_context(tc.tile_pool(name="p_h3", bufs=1))
                h3 = layernorm(x3T, p_h3, "ln3")
                gp = stf.enter_context(tc.tile_pool(name="p_g", bufs=2))
                ps = stf.enter_context(tc.tile_pool(name="ps_ff1", bufs=3,
                                                    space="PSUM"))
                for i in range(32):
                    wt = ff1_tiles[i]
                    psg = ps.tile([128, TO], F32, name="psg", tag="p")
                    for kt in range(CKT):
                        nc.tensor.matmul(psg[:], wt[:, 0, kt, :], h3[kt][:],
                                         start=(kt == 0), stop=(kt == CKT - 1))
                    g = gp.tile([128, TO], F32, name="g", tag="g")
                    nc.scalar.activation(g[:], psg[:], AF.Gelu,
                                         bias=fb1[:, 32 + i:33 + i], scale=1.0)
                    psa = ps.tile([128, TO], F32, name="psa", tag="p")
                    for kt in range(CKT):
                        nc.tensor.matmul(psa[:], wt[:, 1, kt, :], h3[kt][:],
                                         start=(kt == 0), stop=(kt == CKT - 1))
                    h = p_hT.tile([128, TO], BF16, name=f"hT{i}")
                    nc.vector.scalar_tensor_tensor(h[:], psa[:], fb1[:, i:i + 1],
                                                   g[:], op0=OP.add, op1=OP.mult)
                    hT.append(h)
                    # stream in the next FF1 pair
                    if i + 8 < 32:
                        wt2 = p_ff1w.tile([128, 2, CKT, 128], BF16,
                                          name="wff1", tag="w")
                        nc.scalar.dma_start(out=wt2, in_=ff1t.ap()[i + 8])
                        ff1_tiles[i + 8] = wt2
                    if i == 0:
                        # kick off FF2 prefetch
                        ff2_tiles = []
                        for m in range(2):
                            w2 = p_ff2w.tile([128, FF // 128, 128], BF16,
                                             name="wff2", tag="w")
                            nc.scalar.dma_start(out=w2, in_=ff2t.ap()[m])
                            ff2_tiles.append(w2)

            with ExitStack() as stf2:
                outp = stf2.enter_context(tc.tile_pool(name="p_out", bufs=2))
                ps = stf2.enter_context(tc.tile_pool(name="ps_ff2", bufs=2,
                                                     space="PSUM"))
                for m in range(8):
                    wm = ff2_tiles[m % 2] if m < 2 else None
                    if m >= 2:
                        wm = p_ff2w.tile([128, FF // 128, 128], BF16,
                                         name="wff2", tag="w")
                        nc.scalar.dma_start(out=wm, in_=ff2t.ap()[m])
                    psy = ps.tile([128, TO], F32, name="psy_ff2", tag="y")
                    for kt in range(FF // 128):
                        nc.tensor.matmul(psy[:], wm[:, kt, :], hT[kt][:],
                                         start=(kt == 0), stop=(kt == FF // 128 - 1))
                    o = outp.tile([128, TO], F32, name="of", tag="of")
                    nc.vector.scalar_tensor_tensor(o[:], psy[:], ff2b[:, m:m + 1],
                                                   x3T[m].bitcast(F32),
                                                   op0=OP.add, op1=OP.add)
                    nc.sync.dma_start(out=outT.ap()[m * 128:(m + 1) * 128, :],
                                      in_=o[:])

    return nc


# ---------------------------------------------------------------------------
# host side
# ---------------------------------------------------------------------------
def _tile_lhs_pairs(w, nm, nkt):
    """[K, M] -> [nm/2, 128, 2, nkt, 128]; [mb][p][j][kt][n] = w[kt*128+p, (2mb+j)*128+n]."""
    K, M = w.shape
    assert K == nkt * 128 and M == nm * 128 and nm % 2 == 0
    return np.ascontiguousarray(
        w.reshape(nkt, 128, nm // 2, 2, 128).transpose(2, 1, 3, 0, 4))


def _tile_lhs(w, nm, nkt):
    """[K, M] -> [nm, 128, nkt, 128]."""
    K, M = w.shape
    assert K == nkt * 128 and M == nm * 128
    return np.ascontiguousarray(
        w.reshape(nkt, 128, nm, 128).transpose(2, 1, 0, 3))


def _tile_ff1_pairs(w):
    """[1024, 8192] -> [32, 128, 2, 8, 128]; [i][p][0]=gate tile 32+i, [i][p][1]=a tile i."""
    t = _tile_lhs(w, 64, 8)           # [64, 128, 8, 128]
    return np.ascontiguousarray(
        np.stack([t[32:64], t[0:32]], axis=2))


def _tile_rhs(w, nkt):
    K, N = w.shape
    assert K == nkt * 128 and N % 512 == 0
    return np.ascontiguousarray(
        w.reshape(nkt, 128, N // 512, 512).transpose(2, 1, 0, 3))


def _bias_cols(b, ncols):
    return np.ascontiguousarray(np.asarray(b, np.float32).reshape(ncols, 128).T)


_NC_CACHE = None


def kernel(**inputs):
    global _NC_CACHE
    inp = {k: np.asarray(v, np.float32) for k, v in inputs.items()}

    x = inp["x"][0]                    # [T, DIM]
    ctx = inp["context"][0]            # [77, CTX]
    xT_full = np.ascontiguousarray(x.T)
    ctxT = np.zeros((CTX, TCXP), np.float32)
    ctxT[:, :TCX] = ctx.T

    wq1 = np.ascontiguousarray((inp["n1_w"][:, None] * inp["q1_w"]) * SCALE)
    wk1 = np.ascontiguousarray(inp["n1_w"][:, None] * inp["k1_w"])
    wv1 = np.ascontiguousarray(inp["n1_w"][:, None] * inp["v1_w"])
    qb1 = (inp["n1_b"] @ inp["q1_w"]) * SCALE
    kb1 = inp["n1_b"] @ inp["k1_w"]
    vb1 = inp["n1_b"] @ inp["v1_w"]
    wq2 = np.ascontiguousarray((inp["n2_w"][:, None] * inp["q2_w"]) * SCALE)
    qb2 = (inp["n2_b"] @ inp["q2_w"]) * SCALE
    ff1 = np.ascontiguousarray(inp["n3_w"][:, None] * inp["ff1_w"])
    fb1 = inp["n3_b"] @ inp["ff1_w"] + inp["ff1_b"]

    sel16 = np.zeros((16, 8, 128), np.float32)
    for p in range(PAIRS):
        sel16[2 * p, p, 0:64] = 1.0
        sel16[2 * p + 1, p, 64:128] = 1.0

    shared = {
        "ctxT": ctxT,
        "sel16": sel16,
        "wq1t": _tile_lhs_pairs(wq1, 8, CKT),
        "wk1t": _tile_lhs_pairs(wk1, 8, CKT),
        "wv1t": _tile_rhs(wv1, CKT),
        "o1t": _tile_lhs_pairs(np.ascontiguousarray(inp["o1_w"]), 8, CKT),
        "wq2t": _tile_lhs_pairs(wq2, 8, CKT),
        "k2t": _tile_lhs_pairs(np.ascontiguousarray(inp["k2_w"]), 8, CKT_CTX),
        "v2t": _tile_rhs(np.ascontiguousarray(inp["v2_w"]), CKT_CTX),
        "o2t": _tile_lhs_pairs(np.ascontiguousarray(inp["o2_w"]), 8, CKT),
        "ff1t": _tile_ff1_pairs(ff1),
        "ff2t": _tile_lhs(np.ascontiguousarray(inp["ff2_w"]), 8, FF // 128),
        "vb1r": np.ascontiguousarray(vb1.reshape(1, DIM)),
    }
    f32_shared = {
        "qb1c": _bias_cols(qb1, 8),
        "kb1c": _bias_cols(kb1, 8),
        "o1bc": _bias_cols(inp["o1_b"], 8),
        "qb2c": _bias_cols(qb2, 8),
        "o2bc": _bias_cols(inp["o2_b"], 8),
        "fb1c": _bias_cols(fb1, 64),
        "ff2bc": _bias_cols(inp["ff2_b"], 8),
        "padmask": np.ascontiguousarray(
            (np.arange(128)[:, None] < TCX).astype(np.float32)
            * np.ones((1, 16), np.float32)),
    }
    shared = {k: np.ascontiguousarray(v, dtype=ml_dtypes.bfloat16)
              for k, v in shared.items()}
    shared.update({k: np.ascontiguousarray(v, dtype=np.float32)
                   for k, v in f32_shared.items()})

    in_maps = []
    for c in range(NCORES):
        m = dict(shared)
        m["xT"] = np.ascontiguousarray(xT_full[:, c * TO:(c + 1) * TO])
        m["rank_in"] = np.full((1, 1), float(c), np.float32)
        in_maps.append(m)

    if _NC_CACHE is None:
        _NC_CACHE = build_nc()
    nc = _NC_CACHE

    res = run_bass_kernel_spmd(nc, in_maps, core_ids=list(range(NCORES)))

    outs = [res.results[c]["outT"].T for c in range(NCORES)]   # each [TO, DIM]
    return np.ascontiguousarray(np.concatenate(outs, axis=0))[None].astype(np.float32)


if __name__ == "__main__":
    d = np.load("/tmp/ref_inputs.npz")
    out = kernel(**{k: d[k] for k in d.files})
    ref = np.load("/tmp/ref_out.npy")
    err = np.abs(out - ref).max()
    print("max abs err:", err, " absmax ref:", np.abs(ref).max(),
          " rel:", err / np.abs(ref).max())
